# revision 1
# baseline (speedup 1.0000x reference)
"""DGCNN Bass kernel for trn2 — per-core builder + host-side folding.

Per core (one sample, N points, k=40 neighbors):
  1. kNN scores via K=4 matmul (s_ij = x_i.x_j - 0.5|x_j|^2; row-affine
     equivalent to the reference's pairwise -dist^2).
  2. top-40 selection on DVE (max8 / max_index / match_replace rounds).
  3. EdgeConv rounds with gather-after-matmul factorization:
     conv([nbr-ctr, ctr]) = A[:, j] + B[:, i], A/B per-point tables.
  4. Global-max head with W7 split (g-part reduces to a per-channel bias).

BN scales folded into weights on host; LeakyReLU commutes with the k/N max
reductions (positive BN scale asserted host-side).
"""
import numpy as np
import concourse.bass as bass
import concourse.mybir as mybir
from concourse.bacc import Bacc
from concourse.tile import TileContext

F32 = mybir.dt.float32
F32R = mybir.dt.float32r
BF16 = mybir.dt.bfloat16
FP16 = mybir.dt.float16
U16 = mybir.dt.uint16
I16 = mybir.dt.int16
AX = mybir.AxisListType
OP = mybir.AluOpType
ACTF = mybir.ActivationFunctionType

KNBR = 40
NEG = -1e30
LEAK = 0.2


def build_core(N=4096, conv_dtype="bf16", sel_mode="mono"):
    nc = Bacc(None)
    T = N // 128
    PAIRS = T // 2
    CD = {"f32": F32, "bf16": BF16}[conv_dtype]

    def din(name, shape, dt=F32):
        return nc.dram_tensor(name, shape, dt, kind="ExternalInput")

    xr_d = din("xr", [4, N], F32R)
    xa_d = din("xa", [4, N], F32R)
    a1w_d = din("a1w", [3, 64], F32R)
    b1w_d = din("b1w", [3, 64], F32R)
    b1_d = din("b1", [64, 1])
    w2t_d = din("w2t", [64, 64], CD)
    b2_d = din("b2", [64, 1])
    a3w_d = din("a3w", [64, 64], F32R)
    b3w_d = din("b3w", [64, 64], F32R)
    b3_d = din("b3", [64, 1])
    w4t_d = din("w4t", [64, 64], CD)
    b4_d = din("b4", [64, 1])
    a5w_d = din("a5w", [64, 64], F32R)
    b5w_d = din("b5w", [64, 64], F32R)
    b5_d = din("b5", [64, 1])
    w6t_d = din("w6t", [64, 3 * 1024], F32R)
    b6_d = din("b6", [128, 8])
    w7gt_d = din("w7gt", [128, 8 * 4 * 128])
    b7_d = din("b7", [128, 4])
    w7xt_d = din("w7xt", [64, 3 * 4 * 128], F32R)
    w8t_d = din("w8t", [128, 4 * 2 * 128], CD)
    b8_d = din("b8", [128, 2])
    w9t_d = din("w9t", [128, 2 * 63], CD)
    b9_d = din("b9", [63, 1])

    out_d = nc.dram_tensor("out", [63, N], F32, kind="ExternalOutput")
    debug = bool(int(__import__("os").environ.get("DGCNN_DEBUG", "0")))
    if debug:
        idx_dbg = nc.dram_tensor("idx_dbg", [128, T * KNBR], U16, kind="ExternalOutput")
        pooled_dbg = nc.dram_tensor("pooled_dbg", [128, T * (N // 8)], F32,
                                    kind="ExternalOutput")
        sbf_dbg = nc.dram_tensor("sbf_dbg", [128, 4 * N], BF16,
                                 kind="ExternalOutput")
        cand_dbg = nc.dram_tensor("cand_dbg", [128, 4 * 328], BF16,
                                  kind="ExternalOutput")
        nbrx_dbg = nc.dram_tensor("nbrx_dbg", [128, T * 48], U16,
                                  kind="ExternalOutput")
        x1_dbg = nc.dram_tensor("x1_dbg", [64, N], F32, kind="ExternalOutput")
        x2_dbg = nc.dram_tensor("x2_dbg", [64, N], F32, kind="ExternalOutput")
        x3_dbg = nc.dram_tensor("x3_dbg", [64, N], F32, kind="ExternalOutput")
        g_dbg = nc.dram_tensor("g_dbg", [128, 8], F32, kind="ExternalOutput")

    with TileContext(nc) as tc:
        with tc.tile_pool(name="persist", bufs=1) as pp:
            # per-pair wrapped edge lists, fully replicated: pair p at cols
            # p*320.., tile 2p in partitions 0-63 (4x16 copies), tile 2p+1
            # in partitions 64-127.
            wraps = pp.tile([128, 320 * (T // 2)], U16)
            x1 = pp.tile([64, N], F32R)
            x2 = pp.tile([64, N], F32R)
            x3 = pp.tile([64, N], F32R)
            nbr_all = (pp.tile([128, T * KNBR], U16)
                       if sel_mode != "fast2" else None)
            if sel_mode == "mono":
                pass
            elif sel_mode == "fast2":
                pass
            else:
                zc = pp.tile([128, 1], F32)
                nc.gpsimd.memset(zc[:], 0.0)
                iotaJ = pp.tile([128, N], U16)
                nc.gpsimd.iota(iotaJ[:], pattern=[[1, N]], base=0,
                               channel_multiplier=0)
                iota256f = pp.tile([128, 256], F32)
                nc.gpsimd.iota(iota256f[:], pattern=[[1, 256]], base=0,
                               channel_multiplier=0,
                               allow_small_or_imprecise_dtypes=True)
                iotaR1 = pp.tile([128, KNBR], U16)
                nc.gpsimd.iota(iotaR1[:], pattern=[[1, KNBR]], base=1,
                               channel_multiplier=0)

            # =====================================================
            # Stage A: kNN + top-40 per tile (monolithic rounds)
            # (ec1 pool + xp pool opened around it: LIFO scoping)
            # =====================================================
            ec1p = tc.tile_pool(name="ec1", bufs=1)
            ecp1 = ec1p.__enter__()
            w2t = ecp1.tile([128, 64], CD, name="w2t")
            nc.sync.dma_start(out=w2t[0:64, :], in_=w2t_d[:])
            nc.sync.dma_start(out=w2t[64:128, :], in_=w2t_d[:])
            xp = tc.tile_pool(name="xp", bufs=1)
            xpp = xp.__enter__()
            xr = xpp.tile([4, N], F32R, name="xr")
            nc.sync.dma_start(out=xr[:], in_=xr_d[:])
            xa = xpp.tile([4, N], F32R, name="xa")
            nc.sync.dma_start(out=xa[:], in_=xa_d[:])
            if sel_mode == "fast2":
                scp = tc.tile_pool(name="selconst", bufs=1)
                scpp = scp.__enter__()
                zc = scpp.tile([128, 1], F32)
                nc.gpsimd.memset(zc[:], 0.0)
                iotaW = scpp.tile([128, N // 8], U16)
                nc.gpsimd.iota(iotaW[:], pattern=[[1, N // 8]], base=0,
                               channel_multiplier=0)
                iotaE40 = scpp.tile([128, 8 * KNBR], U16)
                nc.gpsimd.iota(iotaE40[:], pattern=[[1, 8], [0, KNBR]], base=0,
                               channel_multiplier=0)
            with tc.tile_pool(name="sel_sb", bufs=2) as sp, \
                 tc.tile_pool(name="sel_ps", bufs=2, space="PSUM") as sps:
                W = N // 8
                for t in range(T):
                    if sel_mode == "fast2":
                        # e-major score planes + shifted bf16 copy
                        s_sb = sp.tile([128, N], F32, tag="s_sb", bufs=1)
                        sbf = sp.tile([128, N], BF16, tag="sbf", bufs=1)
                        pooled = sp.tile([128, W], F32, tag="pooled", bufs=1)
                    else:
                        s_sb = sp.tile([128, N], F32, tag="s_sb", bufs=1)
                        if sel_mode != "mono":
                            pooled = sp.tile([128, N // 8], F32, tag="pooled",
                                             bufs=1)
                            sbf = sp.tile([128, N], BF16, tag="sbf", bufs=1)
                    with nc.named_scope("knn_score"):
                        for h in range(2):
                            ps = sps.tile([128, N // 2], F32, tag="score")
                            for j in range(N // 2 // 512):
                                col = h * (N // 2) + j * 512
                                nc.tensor.matmul(
                                    ps[:, j * 512:(j + 1) * 512],
                                    xa[:, t * 128:(t + 1) * 128],
                                    xr[:, col:col + 512],
                                    start=True, stop=True)
                            if sel_mode == "fast2":
                                nc.scalar.copy(
                                    out=s_sb.rearrange("p (e w) -> p e w", e=8)
                                    [:, :, h * (W // 2):(h + 1) * (W // 2)],
                                    in_=ps.rearrange("p (w e) -> p e w", e=8))
                            else:
                                nc.scalar.copy(
                                    out=s_sb[:, h * (N // 2):(h + 1) * (N // 2)],
                                    in_=ps[:])
                            if sel_mode != "mono":
                                nc.vector.tensor_reduce(
                                    out=pooled[:, h * (W // 2):(h + 1) * (W // 2)],
                                    in_=ps.rearrange("p (w k) -> p w k", k=8),
                                    axis=AX.X, op=OP.max)
                    sel_scope = nc.named_scope("sel")
                    sel_scope.__enter__()
                    m8 = sp.tile([128, 8], F32, tag="m8")
                    if sel_mode == "mono":
                        for r in range(5):
                            nc.vector.max(out=m8[:], in_=s_sb[:])
                            nc.vector.max_index(
                                out=nbr_all[:, t * KNBR + r * 8: t * KNBR + (r + 1) * 8],
                                in_max=m8[:], in_values=s_sb[:])
                            nc.vector.match_replace(out=s_sb[:], in_to_replace=m8[:],
                                                    in_values=s_sb[:], imm_value=NEG)
                    elif sel_mode == "fast2":
                        # --- extract top-40 windows (exactly 40, 8 elems each)
                        for r in range(5):
                            nc.vector.max(out=m8[:], in_=pooled[:])
                            nc.vector.match_replace(
                                out=pooled[:], in_to_replace=m8[:],
                                in_values=pooled[:], imm_value=NEG)
                        negtau = sp.tile([128, 1], F32, tag="negtau")
                        nc.vector.tensor_scalar(negtau[:], m8[:, 7:8], -1.0,
                                                None, op0=OP.mult)
                        nc.scalar.activation(sbf[:], s_sb[:], ACTF.Identity,
                                             bias=negtau[:])
                        wmask = sp.tile([128, W], FP16, tag="wmask")
                        nc.vector.tensor_scalar(wmask[:], pooled[:], -1e29, None,
                                                op0=OP.is_le)
                        wrank = sp.tile([128, W], F32, tag="wrank")
                        nc.vector.tensor_tensor_scan(
                            wrank[:], wmask[:], zc.broadcast_to([128, W]), 0.0,
                            op0=OP.add, op1=OP.add)
                        wmm1 = sp.tile([128, W], FP16, tag="wmm1")
                        nc.vector.tensor_scalar(wmm1[:], wmask[:], -1.0, None,
                                                op0=OP.add)
                        wm0 = sp.tile([128, W], FP16, tag="wm0")
                        nc.vector.scalar_tensor_tensor(
                            out=wm0[:], in0=wrank[:], scalar=1.0, in1=wmask[:],
                            op0=OP.mult, op1=OP.mult)
                        # wm1: rank 1..40 at extracted windows, -1 elsewhere
                        wm1 = sp.tile([128, W], I16, tag="wm1")
                        nc.vector.tensor_tensor(out=wm1[:], in0=wm0[:],
                                                in1=wmm1[:], op=OP.add)
                        # window id of each rank (slot r holds window index)
                        winv = sp.tile([128, 48], U16, tag="winv", bufs=3)
                        nc.gpsimd.local_scatter(winv[:], iotaW[:], wm1[:],
                                                channels=128, num_elems=48,
                                                num_idxs=W)
                        # gather the 40 windows' contents: 8 disjoint-slice
                        # scatters sharing the window-rank index wm1, then an
                        # Act compaction to a contiguous candidate array.
                        cand8 = sp.tile([128, 8 * 42], BF16, tag="cand8")
                        for e in range(8):
                            nc.gpsimd.local_scatter(
                                cand8[:, e * 42:(e + 1) * 42],
                                sbf[:, e * W:(e + 1) * W], wm1[:],
                                channels=128, num_elems=42, num_idxs=W)
                        cand = sp.tile([128, 8 * KNBR], BF16, tag="cand")
                        nc.scalar.copy(
                            out=cand.rearrange("p (e r) -> p e r", e=8),
                            in_=cand8.rearrange("p (e r) -> p e r", r=42)
                            [:, :, 1:41])
                        # original j of each cand slot: winv[r]*8 + e
                        winv8 = sp.tile([128, 41], U16, tag="winv8")
                        nc.vector.tensor_scalar(winv8[:], winv[:, 0:41], 8, None,
                                                op0=OP.mult)
                        orig = sp.tile([128, 8 * KNBR], U16, tag="orig")
                        nc.vector.tensor_tensor(
                            out=orig.rearrange("p (e r) -> p e r", e=8),
                            in0=winv8[:, 1:41].unsqueeze(1)
                            .broadcast_to([128, 8, KNBR]),
                            in1=iotaE40.rearrange("p (e r) -> p e r", e=8),
                            op=OP.add)
                        # top-40 of the 320 candidates (mark via match_replace)
                        cv = cand[:, 0:8 * KNBR]
                        for r in range(5):
                            nc.vector.max(out=m8[:], in_=cv)
                            nc.vector.match_replace(out=cv, in_to_replace=m8[:],
                                                    in_values=cv, imm_value=NEG)
                        cmask = sp.tile([128, 8 * KNBR], FP16, tag="cmask")
                        nc.vector.tensor_scalar(cmask[:], cv, -1e29, None,
                                                op0=OP.is_le)
                        crank = sp.tile([128, 8 * KNBR], F32, tag="crank")
                        nc.vector.tensor_tensor_scan(
                            crank[:], cmask[:], zc.broadcast_to([128, 8 * KNBR]),
                            0.0, op0=OP.add, op1=OP.add)
                        cmm1 = sp.tile([128, 8 * KNBR], FP16, tag="cmm1")
                        nc.vector.tensor_scalar(cmm1[:], cmask[:], -1.0, None,
                                                op0=OP.add)
                        cm0 = sp.tile([128, 8 * KNBR], FP16, tag="cm0")
                        nc.vector.scalar_tensor_tensor(
                            out=cm0[:], in0=crank[:], scalar=1.0, in1=cmask[:],
                            op0=OP.mult, op1=OP.mult)
                        cslot = sp.tile([128, 8 * KNBR], I16, tag="cslot")
                        nc.vector.tensor_tensor(out=cslot[:], in0=cm0[:],
                                                in1=cmm1[:], op=OP.add)
                        nbrx = sp.tile([128, 48], U16, tag="nbrx", bufs=3)
                        nc.gpsimd.local_scatter(nbrx[:], orig[:], cslot[:],
                                                channels=128, num_elems=48,
                                                num_idxs=8 * KNBR)
                        if debug:
                            nc.sync.dma_start(
                                out=pooled_dbg[:, t * W:(t + 1) * W],
                                in_=pooled[:])
                            nc.sync.dma_start(
                                out=nbrx_dbg[:, t * 48:(t + 1) * 48],
                                in_=nbrx[:])
                            if t < 4:
                                nc.sync.dma_start(
                                    out=sbf_dbg[:, t * N:(t + 1) * N],
                                    in_=sbf[:])
                                nc.sync.dma_start(
                                    out=cand_dbg[:, t * 328:(t + 1) * 328],
                                    in_=cand[:])
                    else:
                        # screen for tau = 40th-largest pooled window max
                        for r in range(5):
                            nc.vector.max(out=m8[:], in_=pooled[:])
                            nc.vector.match_replace(out=pooled[:], in_to_replace=m8[:],
                                                    in_values=pooled[:], imm_value=NEG)
                        tau = m8[:, 7:8]
                        negtau = sp.tile([128, 1], F32, tag="negtau")
                        nc.vector.tensor_scalar(negtau[:], tau, -1.0, None,
                                                op0=OP.mult)
                        nc.scalar.activation(sbf[:], s_sb[:], ACTF.Identity,
                                             bias=negtau[:])
                        maskf = sp.tile([128, N], BF16, tag="maskf", bufs=1)
                        nc.vector.tensor_scalar(maskf[:], s_sb[:], tau, None,
                                                op0=OP.is_ge)
                        rankf = sp.tile([128, N], F32, tag="rankf", bufs=1)
                        nc.vector.tensor_tensor_scan(
                            rankf[:], maskf[:], zc.broadcast_to([128, N]), 0.0,
                            op0=OP.add, op1=OP.add)
                        slotf = sp.tile([128, N], I16, tag="slotf", bufs=1)
                        nc.vector.scalar_tensor_tensor(
                            out=slotf[:], in0=rankf[:], scalar=1.0, in1=maskf[:],
                            op0=OP.mult, op1=OP.mult)
                        candb = sp.tile([128, 256], BF16, tag="candb", bufs=1)
                        nc.gpsimd.local_scatter(candb[:], sbf[:], slotf[:],
                                                channels=128, num_elems=256,
                                                num_idxs=N)
                        cidx = sp.tile([128, 256], U16, tag="cidx", bufs=1)
                        nc.gpsimd.local_scatter(cidx[:], iotaJ[:], slotf[:],
                                                channels=128, num_elems=256,
                                                num_idxs=N)
                        count = rankf[:, N - 1: N]
                        emptym = sp.tile([128, 256], F32, tag="emptym", bufs=1)
                        nc.vector.tensor_scalar(emptym[:], iota256f[:], count, None,
                                                op0=OP.is_gt)
                        candfix = sp.tile([128, 256], F32, tag="candfix", bufs=1)
                        nc.vector.scalar_tensor_tensor(
                            out=candfix[:], in0=emptym[:], scalar=NEG, in1=candb[:],
                            op0=OP.mult, op1=OP.add)
                        pos40 = sp.tile([128, KNBR], U16, tag="pos40")
                        cv = candfix[:, 1:256]
                        for r in range(5):
                            nc.vector.max(out=m8[:], in_=cv)
                            nc.vector.max_index(out=pos40[:, r * 8:(r + 1) * 8],
                                                in_max=m8[:], in_values=cv)
                            nc.vector.match_replace(out=cv, in_to_replace=m8[:],
                                                    in_values=cv, imm_value=NEG)
                        posi = sp.tile([128, KNBR], I16, tag="posi")
                        nc.vector.tensor_scalar(posi[:], pos40[:], 1, None, op0=OP.add)
                        winv = sp.tile([128, 256], U16, tag="winv", bufs=1)
                        nc.gpsimd.local_scatter(winv[:], iotaR1[:], posi[:],
                                                channels=128, num_elems=256,
                                                num_idxs=KNBR)
                        winm1 = sp.tile([128, 256], I16, tag="winm1", bufs=1)
                        nc.vector.tensor_scalar(winm1[:], winv[:], 1, None,
                                                op0=OP.subtract)
                        nc.gpsimd.local_scatter(
                            nbr_all[:, t * KNBR:(t + 1) * KNBR], cidx[:], winm1[:],
                            channels=128, num_elems=KNBR, num_idxs=256)
                    sel_scope.__exit__(None, None, None)
                    # wrapped list: wrap[ilo, kk*8+ihi] = nbr[ihi*16+ilo, kk]
                    pbase = 64 * (t % 2)
                    cb = (t // 2) * 320
                    for ihi in range(8):
                        dst = wraps[pbase:pbase + 16, cb:cb + 320] \
                            .rearrange("p (k e) -> p k e", e=8)[:, :, ihi:ihi + 1]
                        if sel_mode == "fast2":
                            nsrc = nbrx[ihi * 16:(ihi + 1) * 16, 1:41]
                        else:
                            nsrc = nbr_all[ihi * 16:(ihi + 1) * 16,
                                           t * KNBR:(t + 1) * KNBR]
                        nc.sync.dma_start(out=dst, in_=nsrc.unsqueeze(2))
                    # replicate to the other three 16-partition groups
                    for rep in range(1, 4):
                        nc.sync.dma_start(
                            out=wraps[pbase + 16 * rep: pbase + 16 * (rep + 1),
                                      cb:cb + 320],
                            in_=wraps[pbase: pbase + 16, cb:cb + 320])
            if debug and sel_mode != "fast2":
                nc.sync.dma_start(out=idx_dbg[:], in_=nbr_all[:])

            # =====================================================
            # EdgeConv machinery
            # =====================================================
            def build_tables(aw_d, bw_d, bias_d, src, arep, brep, kdim,
                             bT=None, pools=None):
                with nc.named_scope("tables"):
                    if pools is None:
                        ctx1 = tc.tile_pool(name="tb_sb", bufs=2)
                        ctx2 = tc.tile_pool(name="tb_ps", bufs=2, space="PSUM")
                        tsp, tps = ctx1.__enter__(), ctx2.__enter__()
                    else:
                        ctx1 = ctx2 = None
                        tsp, tps = pools
                    awt = tsp.tile([kdim, 64], F32R, tag="awt")
                    bwt = tsp.tile([kdim, 64], F32R, tag="bwt")
                    nc.sync.dma_start(out=awt[:], in_=aw_d[:])
                    nc.sync.dma_start(out=bwt[:], in_=bw_d[:])
                    if bT is None:
                        biast = tsp.tile([64, 1], F32, tag="biast")
                        nc.sync.dma_start(out=biast[:], in_=bias_d[:])
                    for j in range(N // 512):
                        srcj = src[:, j * 512:(j + 1) * 512]
                        psa = tps.tile([64, 512], F32, tag="psa")
                        nc.tensor.matmul(psa[:], awt[:], srcj,
                                         start=True, stop=True)
                        nc.scalar.copy(out=arep[0:64, j * 512:(j + 1) * 512], in_=psa[:])
                        if bT is None:
                            psb = tps.tile([64, 512], F32, tag="psb")
                            nc.tensor.matmul(psb[:], bwt[:], srcj,
                                             start=True, stop=True)
                            nc.scalar.activation(brep[0:64, j * 512:(j + 1) * 512],
                                                 psb[:], ACTF.Identity,
                                                 bias=biast[:])
                    if bT is not None:
                        # transposed b-tables: bT[i, c-of-tile-t] per point i
                        for t_ in range(T):
                            psb = tps.tile([128, 64], F32, tag="psbT")
                            nc.tensor.matmul(psb[:],
                                             src[:, t_ * 128:(t_ + 1) * 128],
                                             bwt[:], start=True, stop=True)
                            nc.scalar.copy(out=bT[:, t_ * 64:(t_ + 1) * 64],
                                           in_=psb[:])
                    nc.sync.dma_start(out=arep[64:128, :], in_=arep[0:64, :])
                    if bT is None:
                        nc.sync.dma_start(out=brep[64:128, 0:N - 128],
                                          in_=brep[0:64, 128:N])
                    if ctx1 is not None:
                        ctx2.__exit__(None, None, None)
                        ctx1.__exit__(None, None, None)

            def edge_round(arep, brep, wt_t, bias_t, xout, last=False,
                           pools=None):
                with nc.named_scope("round"):
                    esp, eps = pools
                    for p in range(PAIRS):
                        tA = 2 * p
                        ga = esp.tile([128, KNBR * 128], F32, tag="ga")
                        nc.gpsimd.ap_gather(ga[:], arep[:],
                                            wraps[:, p * 320:(p + 1) * 320]
                                            .bitcast(I16),
                                            channels=128, num_elems=N, d=1,
                                            num_idxs=KNBR * 128)
                        if last:
                            mx = esp.tile([128, 128], F32, tag="mx")
                            nc.vector.tensor_reduce(
                                out=mx[:], in_=ga.rearrange("p (k i) -> p i k", k=KNBR),
                                axis=AX.X, op=OP.max)
                            zz = esp.tile([128, 128], F32, tag="zz")
                            nc.vector.tensor_tensor(
                                out=zz[:], in0=mx[:],
                                in1=brep[:, tA * 128: tA * 128 + 128], op=OP.add)
                            xo = esp.tile([128, 128], F32R, tag="xo")
                            nc.scalar.activation(xo[:], zz[:], ACTF.Prelu, alpha=LEAK)
                        else:
                            bview = brep[:, tA * 128: tA * 128 + 128] \
                                .unsqueeze(1).broadcast_to([128, KNBR, 128])
                            gab = esp.tile([128, KNBR * 128], BF16, tag="gab")
                            nc.scalar.copy(out=gab[:], in_=ga[:])
                            e = esp.tile([128, KNBR * 128], CD, tag="e", bufs=4)
                            nc.vector.tensor_tensor(
                                out=e.rearrange("p (k i) -> p k i", k=KNBR),
                                in0=gab.rearrange("p (k i) -> p k i", k=KNBR),
                                in1=bview, op=OP.add)
                            nc.scalar.activation(e[:], e[:], ACTF.Prelu,
                                                 alpha=LEAK)
                            NCH = KNBR * 128 // 512
                            pmax = esp.tile([128, NCH * 128], F32, tag="pmax", bufs=1)
                            for c in range(NCH):
                                cps = eps.tile([128, 512], F32, tag="cps", bufs=6)
                                nc.tensor.matmul(cps[0:64, :], wt_t[0:64, :],
                                                 e[0:64, c * 512:(c + 1) * 512],
                                                 start=True, stop=True)
                                nc.tensor.matmul(cps[64:128, :], wt_t[64:128, :],
                                                 e[64:128, c * 512:(c + 1) * 512],
                                                 start=True, stop=True)
                                nc.vector.tensor_reduce(
                                    out=pmax[:, c * 128:(c + 1) * 128],
                                    in_=cps.rearrange("p (k i) -> p i k", k=4),
                                    axis=AX.X, op=OP.max)
                            mx = esp.tile([128, 128], F32, tag="mx")
                            nc.vector.tensor_reduce(
                                out=mx[:], in_=pmax.rearrange("p (k i) -> p i k", k=NCH),
                                axis=AX.X, op=OP.max)
                            xo = esp.tile([128, 128], F32R, tag="xo")
                            nc.scalar.activation(xo[:], mx[:], ACTF.Prelu,
                                                 bias=bias_t[:], alpha=LEAK)
                        nc.sync.dma_start(out=xout[:, tA * 128:(tA + 1) * 128],
                                          in_=xo[0:64, :])
                        nc.sync.dma_start(out=xout[:, (tA + 1) * 128:(tA + 2) * 128],
                                          in_=xo[64:128, :])

            def load_bias128(bias_d_, pool, tag="bias128"):
                bt = pool.tile([128, 1], F32, tag=tag)
                nc.sync.dma_start(out=bt[0:64, :], in_=bias_d_[:])
                nc.sync.dma_start(out=bt[64:128, :], in_=bias_d_[:])
                return bt

            # ---- EdgeConv 1 ----
            if sel_mode == "fast2":
                scp.__exit__(None, None, None)
            xp.__exit__(None, None, None)
            b2r = load_bias128(b2_d, ecp1)
            er_sbp = tc.tile_pool(name="er_sb", bufs=2)
            er_psp = tc.tile_pool(name="er_ps", bufs=2, space="PSUM")
            erpools = (er_sbp.__enter__(), er_psp.__enter__())
            tb_sbp = tc.tile_pool(name="tb_sb", bufs=2)
            tb_psp = tc.tile_pool(name="tb_ps", bufs=1, space="PSUM")
            tbpools = (tb_sbp.__enter__(), tb_psp.__enter__())
            esp0 = erpools[0]
            a1rep = esp0.tile([128, N], F32, tag="arepX", bufs=1)
            b1rep = esp0.tile([128, N], BF16, tag="brepX", bufs=1)
            with tc.tile_pool(name="xap", bufs=1) as xap:
                xa2 = xap.tile([4, N], F32R)
                nc.sync.dma_start(out=xa2[:], in_=xa_d[:])
                build_tables(a1w_d, b1w_d, b1_d, xa2[0:3, :], a1rep, b1rep, 3,
                             pools=tbpools)
            edge_round(a1rep, b1rep, w2t, b2r, x1, pools=erpools)

            # ---- EdgeConv 2 + 3 (shared pool, tag-reused tables) ----
            with tc.tile_pool(name="ec23", bufs=1) as ecp:
                a3rep = esp0.tile([128, N], F32, tag="arepX", bufs=1)
                b3rep = esp0.tile([128, N], BF16, tag="brepX", bufs=1)
                w4t = ecp.tile([128, 64], CD)
                nc.sync.dma_start(out=w4t[0:64, :], in_=w4t_d[:])
                nc.sync.dma_start(out=w4t[64:128, :], in_=w4t_d[:])
                b4r = load_bias128(b4_d, ecp)
                build_tables(a3w_d, b3w_d, b3_d, x1, a3rep, b3rep, 64,
                             pools=tbpools)
                edge_round(a3rep, b3rep, w4t, b4r, x2, pools=erpools)

                a5rep = esp0.tile([128, N], F32, tag="arepX", bufs=1)
                b5rep = esp0.tile([128, N], BF16, tag="brepX", bufs=1)
                build_tables(a5w_d, b5w_d, b5_d, x2, a5rep, b5rep, 64,
                             pools=tbpools)
                edge_round(a5rep, b5rep, None, None, x3, last=True,
                           pools=erpools)

            tb_psp.__exit__(None, None, None)
            tb_sbp.__exit__(None, None, None)
            er_psp.__exit__(None, None, None)
            er_sbp.__exit__(None, None, None)
            ec1p.__exit__(None, None, None)

            # =====================================================
            # Head
            # =====================================================
            with nc.named_scope("head"), \
                 tc.tile_pool(name="hd", bufs=1) as hp, \
                 tc.tile_pool(name="hd_sb", bufs=4) as hsp, \
                 tc.tile_pool(name="hd_ps", bufs=4, space="PSUM") as hps:
                w6t = hp.tile([64, 3 * 1024], F32R)
                nc.sync.dma_start(out=w6t[:], in_=w6t_d[:])
                b6t = hp.tile([128, 8], F32)
                nc.sync.dma_start(out=b6t[:], in_=b6_d[:])
                w7gt = hp.tile([128, 8 * 4 * 128], F32)
                nc.sync.dma_start(out=w7gt[:], in_=w7gt_d[:])
                b7t = hp.tile([128, 4], F32)
                nc.sync.dma_start(out=b7t[:], in_=b7_d[:])
                w7xt = hp.tile([64, 3 * 4 * 128], F32R)
                nc.sync.dma_start(out=w7xt[:], in_=w7xt_d[:])
                w8t = hp.tile([128, 4 * 2 * 128], CD)
                nc.sync.dma_start(out=w8t[:], in_=w8t_d[:])
                b8t = hp.tile([128, 2], F32)
                nc.sync.dma_start(out=b8t[:], in_=b8_d[:])
                w9t = hp.tile([128, 2 * 63], CD)
                nc.sync.dma_start(out=w9t[:], in_=w9t_d[:])
                b9t = hp.tile([63, 1], F32)
                nc.sync.dma_start(out=b9t[:], in_=b9_d[:])

                if debug:
                    nc.sync.dma_start(out=x1_dbg[:], in_=x1.bitcast(F32)[:, :])
                    nc.sync.dma_start(out=x2_dbg[:], in_=x2.bitcast(F32)[:, :])
                    nc.sync.dma_start(out=x3_dbg[:], in_=x3.bitcast(F32)[:, :])
                xs_ = [x1, x2, x3]
                NC6 = N // 512
                gtmp = hp.tile([128, 8 * NC6], F32)
                for o in range(8):
                    for n in range(NC6):
                        ps6 = hps.tile([128, 512], F32, tag="hps")
                        for kp in range(3):
                            nc.tensor.matmul(
                                ps6[:],
                                w6t[:, kp * 1024 + o * 128: kp * 1024 + (o + 1) * 128],
                                xs_[kp][:, n * 512:(n + 1) * 512],
                                start=(kp == 0), stop=(kp == 2))
                        nc.vector.tensor_reduce(
                            out=gtmp[:, o * NC6 + n: o * NC6 + n + 1],
                            in_=ps6[:], axis=AX.X, op=OP.max)
                g = hp.tile([128, 8], F32)
                nc.vector.tensor_reduce(
                    out=g[:], in_=gtmp.rearrange("p (o n) -> p o n", o=8),
                    axis=AX.X, op=OP.max)
                nc.vector.tensor_tensor(out=g[:], in0=g[:], in1=b6t[:], op=OP.add)
                g2 = hp.tile([128, 8], F32)
                nc.vector.scalar_tensor_tensor(
                    out=g2[:], in0=g[:], scalar=LEAK, in1=g[:],
                    op0=OP.mult, op1=OP.max)
                if debug:
                    nc.sync.dma_start(out=g_dbg[:], in_=g2[:])

                ps7v = hps.tile([128, 4], F32, tag="ps7v", bufs=1)
                for m in range(4):
                    for o in range(8):
                        nc.tensor.matmul(
                            ps7v[:, m:m + 1],
                            w7gt[:, (o * 4 + m) * 128:(o * 4 + m + 1) * 128],
                            g2[:, o:o + 1], start=(o == 0), stop=(o == 7))
                v7 = hp.tile([128, 4], F32)
                nc.vector.tensor_tensor(out=v7[:], in0=ps7v[:], in1=b7t[:], op=OP.add)

                for n in range(NC6):
                    y7 = hsp.tile([128, 4 * 512], CD, tag="y7", bufs=2)
                    for m in range(4):
                        ps7 = hps.tile([128, 512], F32, tag="hps")
                        for kp in range(3):
                            nc.tensor.matmul(
                                ps7[:],
                                w7xt[:, (kp * 4 + m) * 128:(kp * 4 + m + 1) * 128],
                                xs_[kp][:, n * 512:(n + 1) * 512],
                                start=(kp == 0), stop=(kp == 2))
                        nc.scalar.activation(y7[:, m * 512:(m + 1) * 512], ps7[:],
                                             ACTF.Prelu, bias=v7[:, m:m + 1],
                                             alpha=LEAK)
                    y8 = hsp.tile([128, 2 * 512], CD, tag="y8")
                    for m in range(2):
                        ps8 = hps.tile([128, 512], F32, tag="hps")
                        for k in range(4):
                            nc.tensor.matmul(
                                ps8[:], w8t[:, (k * 2 + m) * 128:(k * 2 + m + 1) * 128],
                                y7[:, k * 512:(k + 1) * 512],
                                start=(k == 0), stop=(k == 3))
                        nc.scalar.activation(y8[:, m * 512:(m + 1) * 512], ps8[:],
                                             ACTF.Prelu, bias=b8t[:, m:m + 1],
                                             alpha=LEAK)
                    ps9 = hps.tile([63, 512], F32, tag="hps")
                    for k in range(2):
                        nc.tensor.matmul(ps9[:], w9t[:, k * 63:(k + 1) * 63],
                                         y8[:, k * 512:(k + 1) * 512],
                                         start=(k == 0), stop=(k == 1))
                    o9 = hsp.tile([63, 512], F32, tag="o9")
                    nc.scalar.activation(o9[:], ps9[:], ACTF.Identity, bias=b9t[:])
                    nc.sync.dma_start(out=out_d[:, n * 512:(n + 1) * 512], in_=o9[:])

    nc.finalize()
    return nc


# =====================================================================
# Host-side folding
# =====================================================================
def fold_weights(inp):
    """inp: the reference setup_inputs() dict. Returns dict of shared
    (sample-independent) device arrays."""
    def f64(a):
        return np.asarray(a, np.float64)

    out = {}
    W1, s1, b1 = f64(inp["W1"]), f64(inp["s1"]), f64(inp["b1"])
    W1a, W1b = W1[:, :3], W1[:, 3:]
    out["a1w"] = (s1[:, None] * W1a).T.astype(np.float32).copy()
    out["b1w"] = (s1[:, None] * (W1b - W1a)).T.astype(np.float32).copy()
    out["b1"] = b1[:, None].astype(np.float32)
    W2, s2, b2 = f64(inp["W2"]), f64(inp["s2"]), f64(inp["b2"])
    assert (s2 > 0).all()
    out["w2t"] = (s2[:, None] * W2).T.astype(np.float32).copy()
    out["b2"] = b2[:, None].astype(np.float32)
    W3, s3, b3 = f64(inp["W3"]), f64(inp["s3"]), f64(inp["b3"])
    W3a, W3b = W3[:, :64], W3[:, 64:]
    out["a3w"] = (s3[:, None] * W3a).T.astype(np.float32).copy()
    out["b3w"] = (s3[:, None] * (W3b - W3a)).T.astype(np.float32).copy()
    out["b3"] = b3[:, None].astype(np.float32)
    W4, s4, b4 = f64(inp["W4"]), f64(inp["s4"]), f64(inp["b4"])
    assert (s4 > 0).all()
    out["w4t"] = (s4[:, None] * W4).T.astype(np.float32).copy()
    out["b4"] = b4[:, None].astype(np.float32)
    W5, s5, b5 = f64(inp["W5"]), f64(inp["s5"]), f64(inp["b5"])
    W5a, W5b = W5[:, :64], W5[:, 64:]
    out["a5w"] = (s5[:, None] * W5a).T.astype(np.float32).copy()
    out["b5w"] = (s5[:, None] * (W5b - W5a)).T.astype(np.float32).copy()
    out["b5"] = b5[:, None].astype(np.float32)
    W6, s6, b6 = f64(inp["W6"]), f64(inp["s6"]), f64(inp["b6"])
    assert (s6 > 0).all()
    W6f = s6[:, None] * W6
    out["w6t"] = W6f.T.reshape(3, 64, 1024).transpose(1, 0, 2) \
        .reshape(64, 3 * 1024).astype(np.float32).copy()
    out["b6"] = b6.reshape(8, 128).T.astype(np.float32).copy()
    W7, s7, b7 = f64(inp["W7"]), f64(inp["s7"]), f64(inp["b7"])
    W7f = s7[:, None] * W7
    W7g, W7x = W7f[:, :1024], W7f[:, 1024:]
    out["w7gt"] = W7g.T.reshape(8, 128, 4, 128).transpose(1, 0, 2, 3) \
        .reshape(128, -1).astype(np.float32).copy()
    out["b7"] = b7.reshape(4, 128).T.astype(np.float32).copy()
    out["w7xt"] = W7x.T.reshape(3, 64, 4, 128).transpose(1, 0, 2, 3) \
        .reshape(64, -1).astype(np.float32).copy()
    W8, s8, b8 = f64(inp["W8"]), f64(inp["s8"]), f64(inp["b8"])
    W8f = s8[:, None] * W8
    out["w8t"] = W8f.T.reshape(4, 128, 2, 128).transpose(1, 0, 2, 3) \
        .reshape(128, -1).astype(np.float32).copy()
    out["b8"] = b8.reshape(2, 128).T.astype(np.float32).copy()
    out["w9t"] = f64(inp["W9"]).T.reshape(2, 128, 63).transpose(1, 0, 2) \
        .reshape(128, 2 * 63).astype(np.float32).copy()
    out["b9"] = f64(inp["b9"])[:, None].astype(np.float32)
    return out


def fold_sample(sample_x):
    """sample_x: (3, N) float32. Returns per-sample arrays."""
    x = np.asarray(sample_x, np.float64)
    xx = (x * x).sum(0)
    N = x.shape[1]
    return {
        "xr": np.concatenate([x, -0.5 * xx[None, :]], 0).astype(np.float32),
        "xa": np.concatenate([x, np.ones((1, N))], 0).astype(np.float32),
    }


def make_in_maps(inputs, n_cores=4):
    """inputs: reference setup_inputs() dict (numpy). One core per sample."""
    shared = fold_weights(inputs)
    x = np.asarray(inputs["x"])
    in_maps = []
    for c in range(n_cores):
        b = c % x.shape[0]
        m = dict(shared)
        m.update(fold_sample(x[b]))
        in_maps.append(m)
    return in_maps


def cast_inputs(in_maps, nc):
    dts = {}
    for alloc in nc.m.functions[0].allocations:
        if isinstance(alloc, mybir.MemoryLocationSet) and alloc.kind == "ExternalInput":
            dts[alloc.memorylocations[0].name] = mybir.dt.np(alloc.dtype)
    outs = []
    for m in in_maps:
        outs.append({k: np.ascontiguousarray(np.asarray(v).astype(dts[k]))
                     for k, v in m.items() if k in dts})
    return outs


# =====================================================================
# Harness entry point
# =====================================================================
_CACHE = {}


def _make_runner(nc, n_cores):
    """Compile-once SPMD runner (mirrors bass2jax.run_bass_via_pjrt but
    caches the jitted executable across kernel() calls)."""
    import jax
    from concourse import bass2jax
    from concourse.bass2jax import _bass_exec_p, partition_id_tensor, \
        install_neuronx_cc_hook

    install_neuronx_cc_hook()
    partition_name = nc.partition_id_tensor.name if nc.partition_id_tensor else None
    in_names, out_names, out_avals, zero_shapes = [], [], [], []
    for alloc in nc.m.functions[0].allocations:
        if not isinstance(alloc, mybir.MemoryLocationSet):
            continue
        name = alloc.memorylocations[0].name
        if alloc.kind == "ExternalInput":
            if name != partition_name:
                in_names.append(name)
        elif alloc.kind == "ExternalOutput":
            shape = tuple(alloc.tensor_shape)
            dtype = mybir.dt.np(alloc.dtype)
            out_names.append(name)
            out_avals.append(jax.core.ShapedArray(shape, dtype))
            zero_shapes.append((shape, dtype))
    n_params = len(in_names)
    all_names = in_names + out_names + ([partition_name] if partition_name else [])
    donate = tuple(range(n_params, n_params + len(out_names)))

    def _body(*args):
        operands = list(args)
        if partition_name is not None:
            operands.append(partition_id_tensor())
        return tuple(_bass_exec_p.bind(
            *operands, out_avals=tuple(out_avals), in_names=tuple(all_names),
            out_names=tuple(out_names), lowering_input_output_aliases=(),
            sim_require_finite=True, sim_require_nnan=True, nc=nc))

    from jax.experimental.shard_map import shard_map
    from jax.sharding import Mesh, PartitionSpec
    mesh = Mesh(np.asarray(jax.devices()[:n_cores]), ("core",))
    in_specs = (PartitionSpec("core"),) * (n_params + len(out_names))
    out_specs = (PartitionSpec("core"),) * len(out_names)
    jf = jax.jit(
        shard_map(_body, mesh=mesh, in_specs=in_specs, out_specs=out_specs,
                  check_rep=False),
        donate_argnums=donate, keep_unused=True)

    import hashlib
    dev_cache = {}

    def _zeros_dev():
        return [jax.numpy.zeros((n_cores * shape[0],) + shape[1:], dtype)
                for shape, dtype in zero_shapes]

    def run(in_maps):
        h = hashlib.md5()
        for name in in_names:
            for m in in_maps:
                h.update(np.asarray(m[name]).tobytes())
        key = h.hexdigest()
        if key not in dev_cache:
            dev_cache.clear()
            arrs = [np.concatenate([np.asarray(m[name]) for m in in_maps], axis=0)
                    for name in in_names]
            dev_cache[key] = [jax.device_put(a) for a in arrs]
        args = list(dev_cache[key]) + _zeros_dev()
        outs = jf(*args)
        return [{n: np.asarray(outs[i]).reshape((n_cores,) + zero_shapes[i][0])[c]
                 for i, n in enumerate(out_names)}
                for c in range(n_cores)]

    return run


def kernel(**inputs):
    """DGCNN forward. inputs keyed as reference.setup_inputs(); returns
    (B, 63, N) float32. Data-parallel: one NeuronCore per sample."""
    from concourse.bass_utils import run_bass_kernel_spmd

    x = np.asarray(inputs["x"])
    B, _, N = x.shape
    key = (B, N)
    if key not in _CACHE:
        nc = build_core(N=N, conv_dtype="bf16", sel_mode="fast2")
        runner = None
        try:
            runner = _make_runner(nc, B)
        except Exception:
            runner = None
        _CACHE[key] = (nc, runner)
    nc, runner = _CACHE[key]
    in_maps = cast_inputs(make_in_maps(inputs, n_cores=B), nc)
    if runner is not None:
        try:
            results = runner(in_maps)
            return np.stack([results[b]["out"] for b in range(B)]).astype(np.float32)
        except Exception:
            _CACHE[key] = (nc, None)
    res = run_bass_kernel_spmd(nc, in_maps, core_ids=list(range(B)))
    return np.stack([res.results[b]["out"] for b in range(B)]).astype(np.float32)



# revision 18
# speedup vs baseline: 1.0932x; 1.0932x over previous
"""DGCNN Bass kernel for trn2 — per-core builder + host-side folding.

Per core (one sample, N points, k=40 neighbors):
  1. kNN scores via K=4 matmul (s_ij = x_i.x_j - 0.5|x_j|^2; row-affine
     equivalent to the reference's pairwise -dist^2).
  2. top-40 selection on DVE (max8 / max_index / match_replace rounds).
  3. EdgeConv rounds with gather-after-matmul factorization:
     conv([nbr-ctr, ctr]) = A[:, j] + B[:, i], A/B per-point tables.
  4. Global-max head with W7 split (g-part reduces to a per-channel bias).

BN scales folded into weights on host; LeakyReLU commutes with the k/N max
reductions (positive BN scale asserted host-side).
"""
import numpy as np
import concourse.bass as bass
import concourse.mybir as mybir
from concourse.bacc import Bacc
from concourse.tile import TileContext

F32 = mybir.dt.float32
F32R = mybir.dt.float32r
BF16 = mybir.dt.bfloat16
FP16 = mybir.dt.float16
U16 = mybir.dt.uint16
I16 = mybir.dt.int16
AX = mybir.AxisListType
OP = mybir.AluOpType
ACTF = mybir.ActivationFunctionType

KNBR = 40
NEG = -1e30
LEAK = 0.2


def build_core(N=4096, conv_dtype="bf16", sel_mode="mono"):
    nc = Bacc(None)
    T = N // 128
    PAIRS = T // 2
    CD = {"f32": F32, "bf16": BF16, "fp16": FP16}[conv_dtype]

    def din(name, shape, dt=F32):
        return nc.dram_tensor(name, shape, dt, kind="ExternalInput")

    xr_d = din("xr", [4, N], F32R)
    xa_d = din("xa", [4, N], F32R)
    a1w_d = din("a1w", [3, 64], F32R)
    b1w_d = din("b1w", [3, 64], F32R)
    b1_d = din("b1", [64, 1])
    w2t_d = din("w2t", [64, 64], CD)
    b2_d = din("b2", [64, 1])
    a3w_d = din("a3w", [64, 64], F32R)
    b3w_d = din("b3w", [64, 64], F32R)
    b3_d = din("b3", [64, 1])
    w4t_d = din("w4t", [64, 64], CD)
    b4_d = din("b4", [64, 1])
    a5w_d = din("a5w", [64, 64], F32R)
    b5w_d = din("b5w", [64, 64], F32R)
    b5_d = din("b5", [64, 1])
    w6t_d = din("w6t", [64, 3 * 1024], F32R)
    b6_d = din("b6", [128, 8])
    w7gt_d = din("w7gt", [128, 8 * 4 * 128])
    b7_d = din("b7", [128, 4])
    w7xt_d = din("w7xt", [64, 3 * 4 * 128], F32R)
    w8t_d = din("w8t", [128, 4 * 2 * 128], CD)
    b8_d = din("b8", [128, 2])
    w9t_d = din("w9t", [128, 2 * 63], CD)
    b9_d = din("b9", [63, 1])

    out_d = nc.dram_tensor("out", [63, N], F32, kind="ExternalOutput")
    debug = bool(int(__import__("os").environ.get("DGCNN_DEBUG", "0")))
    if debug:
        idx_dbg = nc.dram_tensor("idx_dbg", [128, T * KNBR], U16, kind="ExternalOutput")
        pooled_dbg = nc.dram_tensor("pooled_dbg", [128, T * (N // 8)], F32,
                                    kind="ExternalOutput")
        sbf_dbg = nc.dram_tensor("sbf_dbg", [128, 4 * N], FP16,
                                 kind="ExternalOutput")
        cand_dbg = nc.dram_tensor("cand_dbg", [128, 4 * 328], FP16,
                                  kind="ExternalOutput")
        nbrx_dbg = nc.dram_tensor("nbrx_dbg", [128, T * 48], U16,
                                  kind="ExternalOutput")
        x1_dbg = nc.dram_tensor("x1_dbg", [64, N], F32, kind="ExternalOutput")
        x2_dbg = nc.dram_tensor("x2_dbg", [64, N], F32, kind="ExternalOutput")
        x3_dbg = nc.dram_tensor("x3_dbg", [64, N], F32, kind="ExternalOutput")
        g_dbg = nc.dram_tensor("g_dbg", [128, 8], F32, kind="ExternalOutput")

    with TileContext(nc) as tc:
        with tc.tile_pool(name="persist", bufs=1) as pp:
            # per-pair wrapped edge lists, fully replicated: pair p at cols
            # p*320.., tile 2p in partitions 0-63 (4x16 copies), tile 2p+1
            # in partitions 64-127.
            wraps = pp.tile([128, 320 * (T // 2)], U16)
            x1 = pp.tile([64, N], F32R)
            x2 = pp.tile([64, N], F32R)
            x3 = pp.tile([64, N], F32R)
            nbr_all = (pp.tile([128, T * KNBR], U16)
                       if sel_mode != "fast2" else None)
            if sel_mode == "mono":
                pass
            elif sel_mode == "fast2":
                pass
            else:
                zc = pp.tile([128, 1], F32)
                nc.gpsimd.memset(zc[:], 0.0)
                iotaJ = pp.tile([128, N], U16)
                nc.gpsimd.iota(iotaJ[:], pattern=[[1, N]], base=0,
                               channel_multiplier=0)
                iota256f = pp.tile([128, 256], F32)
                nc.gpsimd.iota(iota256f[:], pattern=[[1, 256]], base=0,
                               channel_multiplier=0,
                               allow_small_or_imprecise_dtypes=True)
                iotaR1 = pp.tile([128, KNBR], U16)
                nc.gpsimd.iota(iotaR1[:], pattern=[[1, KNBR]], base=1,
                               channel_multiplier=0)

            # =====================================================
            # Stage A: kNN + top-40 per tile (monolithic rounds)
            # (ec1 pool + xp pool opened around it: LIFO scoping)
            # =====================================================
            ec1p = tc.tile_pool(name="ec1", bufs=1)
            ecp1 = ec1p.__enter__()
            w2t = ecp1.tile([128, 64], CD, name="w2t")
            nc.sync.dma_start(out=w2t[0:64, :], in_=w2t_d[:])
            nc.sync.dma_start(out=w2t[64:128, :], in_=w2t_d[:])
            xp = tc.tile_pool(name="xp", bufs=1)
            xpp = xp.__enter__()
            xr = xpp.tile([4, N], F32R, name="xr")
            nc.sync.dma_start(out=xr[:], in_=xr_d[:])
            xa = xpp.tile([4, N], F32R, name="xa")
            nc.sync.dma_start(out=xa[:], in_=xa_d[:])
            if sel_mode == "fast2":
                scp = tc.tile_pool(name="selconst", bufs=1)
                scpp = scp.__enter__()
                zf = scpp.tile([128, N // 8], FP16)
                nc.gpsimd.memset(zf[:], 0.0)
                iotaW = scpp.tile([128, N // 8], U16)
                nc.gpsimd.iota(iotaW[:], pattern=[[1, N // 8]], base=0,
                               channel_multiplier=0)
                iotaE40 = scpp.tile([128, 8 * KNBR], U16)
                nc.gpsimd.iota(iotaE40[:], pattern=[[1, 8], [0, KNBR]], base=0,
                               channel_multiplier=0)
            with tc.tile_pool(name="sel_sb", bufs=2) as sp, \
                 tc.tile_pool(name="sel_ps", bufs=2, space="PSUM") as sps:
                W = N // 8
                for t in range(T):
                    if sel_mode == "fast2":
                        # e-major fp16 score plane; window maxima via 4x STT tree
                        s_sb = sp.tile([128, N], FP16, tag="s_sb", bufs=2)
                        t1 = sp.tile([128, N // 2], FP16, tag="t1", bufs=2)
                        t2 = sp.tile([128, N // 4], FP16, tag="t2", bufs=2)
                        pooled = sp.tile([128, W], FP16, tag="pooled", bufs=2)
                    else:
                        s_sb = sp.tile([128, N], F32, tag="s_sb", bufs=1)
                        if sel_mode != "mono":
                            pooled = sp.tile([128, N // 8], F32, tag="pooled",
                                             bufs=1)
                            sbf = sp.tile([128, N], BF16, tag="sbf", bufs=1)
                    with nc.named_scope("knn_score"):
                        for h in range(2):
                            ps = sps.tile([128, N // 2], F32, tag="score")
                            for j in range(N // 2 // 512):
                                col = h * (N // 2) + j * 512
                                nc.tensor.matmul(
                                    ps[:, j * 512:(j + 1) * 512],
                                    xa[:, t * 128:(t + 1) * 128],
                                    xr[:, col:col + 512],
                                    start=True, stop=True)
                            if sel_mode == "fast2":
                                nc.scalar.copy(
                                    out=s_sb.rearrange("p (e w) -> p e w", e=8)
                                    [:, :, h * (W // 2):(h + 1) * (W // 2)],
                                    in_=ps.rearrange("p (w e) -> p e w", e=8))
                            else:
                                nc.scalar.copy(
                                    out=s_sb[:, h * (N // 2):(h + 1) * (N // 2)],
                                    in_=ps[:])
                            if sel_mode not in ("mono", "fast2"):
                                nc.vector.tensor_reduce(
                                    out=pooled[:, h * (W // 2):(h + 1) * (W // 2)],
                                    in_=ps.rearrange("p (w k) -> p w k", k=8),
                                    axis=AX.X, op=OP.max)
                            if sel_mode == "fast2":
                                # per-half window-max tree over the 8 e-planes
                                hw = slice(h * (W // 2), (h + 1) * (W // 2))
                                sv = s_sb.rearrange("p (e w) -> p e w", e=8)
                                t1v = t1.rearrange("p (e w) -> p e w", e=4)
                                t2v = t2.rearrange("p (e w) -> p e w", e=2)
                                nc.vector.tensor_tensor(
                                    out=t1v[:, :, hw], in0=sv[:, 0:4, hw],
                                    in1=sv[:, 4:8, hw], op=OP.max)
                                nc.vector.tensor_tensor(
                                    out=t2v[:, :, hw], in0=t1v[:, 0:2, hw],
                                    in1=t1v[:, 2:4, hw], op=OP.max)
                                nc.vector.tensor_tensor(
                                    out=pooled[:, hw], in0=t2v[:, 0, hw],
                                    in1=t2v[:, 1, hw], op=OP.max)
                    sel_scope = nc.named_scope("sel")
                    sel_scope.__enter__()
                    m8 = sp.tile([128, 8], FP16 if sel_mode == "fast2" else F32,
                                 tag="m8")
                    if sel_mode == "mono":
                        for r in range(5):
                            nc.vector.max(out=m8[:], in_=s_sb[:])
                            nc.vector.max_index(
                                out=nbr_all[:, t * KNBR + r * 8: t * KNBR + (r + 1) * 8],
                                in_max=m8[:], in_values=s_sb[:])
                            nc.vector.match_replace(out=s_sb[:], in_to_replace=m8[:],
                                                    in_values=s_sb[:], imm_value=NEG)
                    elif sel_mode == "fast2":
                        # --- extract top-40 windows (exactly 40, 8 elems each)
                        for r in range(5):
                            nc.vector.max(out=m8[:], in_=pooled[:])
                            nc.vector.match_replace(
                                out=pooled[:], in_to_replace=m8[:],
                                in_values=pooled[:], imm_value=NEG)
                        wmask = sp.tile([128, W], FP16, tag="wmask")
                        nc.vector.tensor_scalar(wmask[:], pooled[:], -1e29, None,
                                                op0=OP.is_le)
                        wrank = sp.tile([128, W], FP16, tag="wrank")
                        nc.vector.tensor_tensor_scan(
                            wrank[:], wmask[:], zf[:, 0:W], 0.0,
                            op0=OP.add, op1=OP.add)
                        wmm1 = sp.tile([128, W], FP16, tag="wmm1")
                        nc.vector.tensor_scalar(wmm1[:], wmask[:], -1.0, None,
                                                op0=OP.add)
                        wm0 = sp.tile([128, W], FP16, tag="wm0")
                        nc.vector.tensor_tensor(
                            out=wm0[:], in0=wrank[:], in1=wmask[:], op=OP.mult)
                        # wm1: rank 1..40 at extracted windows, -1 elsewhere
                        wm1 = sp.tile([128, W], I16, tag="wm1")
                        nc.vector.tensor_tensor(out=wm1[:], in0=wm0[:],
                                                in1=wmm1[:], op=OP.add)
                        # window id of each rank (slot r holds window index)
                        winv = sp.tile([128, 48], U16, tag="winv", bufs=3)
                        nc.gpsimd.local_scatter(winv[:], iotaW[:], wm1[:],
                                                channels=128, num_elems=48,
                                                num_idxs=W)
                        # gather the 40 windows' contents: 8 disjoint-slice
                        # scatters sharing the window-rank index wm1, then an
                        # Act compaction to a contiguous candidate array.
                        cand8 = sp.tile([128, 8 * 42], FP16, tag="cand8")
                        for e in range(8):
                            nc.gpsimd.local_scatter(
                                cand8[:, e * 42:(e + 1) * 42],
                                s_sb[:, e * W:(e + 1) * W], wm1[:],
                                channels=128, num_elems=42, num_idxs=W)
                        cand = sp.tile([128, 8 * KNBR], FP16, tag="cand")
                        nc.scalar.copy(
                            out=cand.rearrange("p (e r) -> p e r", e=8),
                            in_=cand8.rearrange("p (e r) -> p e r", r=42)
                            [:, :, 1:41])
                        # original j of each cand slot: winv[r]*8 + e
                        winv8 = sp.tile([128, 41], U16, tag="winv8")
                        nc.vector.tensor_scalar(winv8[:], winv[:, 0:41], 8, None,
                                                op0=OP.mult)
                        orig = sp.tile([128, 8 * KNBR], U16, tag="orig")
                        nc.vector.tensor_tensor(
                            out=orig.rearrange("p (e r) -> p e r", e=8),
                            in0=winv8[:, 1:41].unsqueeze(1)
                            .broadcast_to([128, 8, KNBR]),
                            in1=iotaE40.rearrange("p (e r) -> p e r", e=8),
                            op=OP.add)
                        # top-40 of the 320 candidates (mark via match_replace)
                        cv = cand[:, 0:8 * KNBR]
                        for r in range(5):
                            nc.vector.max(out=m8[:], in_=cv)
                            nc.vector.match_replace(out=cv, in_to_replace=m8[:],
                                                    in_values=cv, imm_value=NEG)
                        cmask = sp.tile([128, 8 * KNBR], FP16, tag="cmask")
                        nc.vector.tensor_scalar(cmask[:], cv, -1e29, None,
                                                op0=OP.is_le)
                        crank = sp.tile([128, 8 * KNBR], FP16, tag="crank")
                        nc.vector.tensor_tensor_scan(
                            crank[:], cmask[:], zf[:, 0:8 * KNBR],
                            0.0, op0=OP.add, op1=OP.add)
                        cmm1 = sp.tile([128, 8 * KNBR], FP16, tag="cmm1")
                        nc.vector.tensor_scalar(cmm1[:], cmask[:], -1.0, None,
                                                op0=OP.add)
                        cm0 = sp.tile([128, 8 * KNBR], FP16, tag="cm0")
                        nc.vector.tensor_tensor(
                            out=cm0[:], in0=crank[:], in1=cmask[:], op=OP.mult)
                        cslot = sp.tile([128, 8 * KNBR], I16, tag="cslot")
                        nc.vector.tensor_tensor(out=cslot[:], in0=cm0[:],
                                                in1=cmm1[:], op=OP.add)
                        nbrx = sp.tile([128, 48], U16, tag="nbrx", bufs=3)
                        nc.gpsimd.local_scatter(nbrx[:], orig[:], cslot[:],
                                                channels=128, num_elems=48,
                                                num_idxs=8 * KNBR)
                        if debug:
                            nc.sync.dma_start(
                                out=pooled_dbg[:, t * W:(t + 1) * W],
                                in_=pooled[:])
                            nc.sync.dma_start(
                                out=nbrx_dbg[:, t * 48:(t + 1) * 48],
                                in_=nbrx[:])
                            if t < 4:
                                nc.sync.dma_start(
                                    out=sbf_dbg[:, t * N:(t + 1) * N],
                                    in_=s_sb[:])
                                nc.sync.dma_start(
                                    out=cand_dbg[:, t * 328:(t + 1) * 328],
                                    in_=cand[:])
                    else:
                        # screen for tau = 40th-largest pooled window max
                        for r in range(5):
                            nc.vector.max(out=m8[:], in_=pooled[:])
                            nc.vector.match_replace(out=pooled[:], in_to_replace=m8[:],
                                                    in_values=pooled[:], imm_value=NEG)
                        tau = m8[:, 7:8]
                        negtau = sp.tile([128, 1], F32, tag="negtau")
                        nc.vector.tensor_scalar(negtau[:], tau, -1.0, None,
                                                op0=OP.mult)
                        nc.scalar.activation(sbf[:], s_sb[:], ACTF.Identity,
                                             bias=negtau[:])
                        maskf = sp.tile([128, N], BF16, tag="maskf", bufs=1)
                        nc.vector.tensor_scalar(maskf[:], s_sb[:], tau, None,
                                                op0=OP.is_ge)
                        rankf = sp.tile([128, N], F32, tag="rankf", bufs=1)
                        nc.vector.tensor_tensor_scan(
                            rankf[:], maskf[:], zc.broadcast_to([128, N]), 0.0,
                            op0=OP.add, op1=OP.add)
                        slotf = sp.tile([128, N], I16, tag="slotf", bufs=1)
                        nc.vector.scalar_tensor_tensor(
                            out=slotf[:], in0=rankf[:], scalar=1.0, in1=maskf[:],
                            op0=OP.mult, op1=OP.mult)
                        candb = sp.tile([128, 256], BF16, tag="candb", bufs=1)
                        nc.gpsimd.local_scatter(candb[:], sbf[:], slotf[:],
                                                channels=128, num_elems=256,
                                                num_idxs=N)
                        cidx = sp.tile([128, 256], U16, tag="cidx", bufs=1)
                        nc.gpsimd.local_scatter(cidx[:], iotaJ[:], slotf[:],
                                                channels=128, num_elems=256,
                                                num_idxs=N)
                        count = rankf[:, N - 1: N]
                        emptym = sp.tile([128, 256], F32, tag="emptym", bufs=1)
                        nc.vector.tensor_scalar(emptym[:], iota256f[:], count, None,
                                                op0=OP.is_gt)
                        candfix = sp.tile([128, 256], F32, tag="candfix", bufs=1)
                        nc.vector.scalar_tensor_tensor(
                            out=candfix[:], in0=emptym[:], scalar=NEG, in1=candb[:],
                            op0=OP.mult, op1=OP.add)
                        pos40 = sp.tile([128, KNBR], U16, tag="pos40")
                        cv = candfix[:, 1:256]
                        for r in range(5):
                            nc.vector.max(out=m8[:], in_=cv)
                            nc.vector.max_index(out=pos40[:, r * 8:(r + 1) * 8],
                                                in_max=m8[:], in_values=cv)
                            nc.vector.match_replace(out=cv, in_to_replace=m8[:],
                                                    in_values=cv, imm_value=NEG)
                        posi = sp.tile([128, KNBR], I16, tag="posi")
                        nc.vector.tensor_scalar(posi[:], pos40[:], 1, None, op0=OP.add)
                        winv = sp.tile([128, 256], U16, tag="winv", bufs=1)
                        nc.gpsimd.local_scatter(winv[:], iotaR1[:], posi[:],
                                                channels=128, num_elems=256,
                                                num_idxs=KNBR)
                        winm1 = sp.tile([128, 256], I16, tag="winm1", bufs=1)
                        nc.vector.tensor_scalar(winm1[:], winv[:], 1, None,
                                                op0=OP.subtract)
                        nc.gpsimd.local_scatter(
                            nbr_all[:, t * KNBR:(t + 1) * KNBR], cidx[:], winm1[:],
                            channels=128, num_elems=KNBR, num_idxs=256)
                    sel_scope.__exit__(None, None, None)
                    # wrapped list: wrap[ilo, kk*8+ihi] = nbr[ihi*16+ilo, kk]
                    pbase = 64 * (t % 2)
                    cb = (t // 2) * 320
                    for ihi in range(8):
                        dst = wraps[pbase:pbase + 16, cb:cb + 320] \
                            .rearrange("p (k e) -> p k e", e=8)[:, :, ihi:ihi + 1]
                        if sel_mode == "fast2":
                            nsrc = nbrx[ihi * 16:(ihi + 1) * 16, 1:41]
                        else:
                            nsrc = nbr_all[ihi * 16:(ihi + 1) * 16,
                                           t * KNBR:(t + 1) * KNBR]
                        nc.sync.dma_start(out=dst, in_=nsrc.unsqueeze(2))
                    # replicate to the other three 16-partition groups
                    for rep in range(1, 4):
                        nc.sync.dma_start(
                            out=wraps[pbase + 16 * rep: pbase + 16 * (rep + 1),
                                      cb:cb + 320],
                            in_=wraps[pbase: pbase + 16, cb:cb + 320])
            if debug and sel_mode != "fast2":
                nc.sync.dma_start(out=idx_dbg[:], in_=nbr_all[:])

            # =====================================================
            # EdgeConv machinery
            # =====================================================
            def build_tables(aw_d, bw_d, bias_d, src, arep, brep, kdim,
                             bT=None, pools=None):
                with nc.named_scope("tables"):
                    if pools is None:
                        ctx1 = tc.tile_pool(name="tb_sb", bufs=2)
                        ctx2 = tc.tile_pool(name="tb_ps", bufs=2, space="PSUM")
                        tsp, tps = ctx1.__enter__(), ctx2.__enter__()
                    else:
                        ctx1 = ctx2 = None
                        tsp, tps = pools
                    awt = tsp.tile([kdim, 64], F32R, tag="awt")
                    bwt = tsp.tile([kdim, 64], F32R, tag="bwt")
                    nc.sync.dma_start(out=awt[:], in_=aw_d[:])
                    nc.sync.dma_start(out=bwt[:], in_=bw_d[:])
                    if bT is None:
                        biast = tsp.tile([64, 1], F32, tag="biast")
                        nc.sync.dma_start(out=biast[:], in_=bias_d[:])
                    for j in range(N // 512):
                        srcj = src[:, j * 512:(j + 1) * 512]
                        psa = tps.tile([64, 512], F32, tag="psa")
                        nc.tensor.matmul(psa[:], awt[:], srcj,
                                         start=True, stop=True)
                        nc.scalar.copy(out=arep[0:64, j * 512:(j + 1) * 512], in_=psa[:])
                        if bT is None:
                            psb = tps.tile([64, 512], F32, tag="psb")
                            nc.tensor.matmul(psb[:], bwt[:], srcj,
                                             start=True, stop=True)
                            nc.scalar.activation(brep[0:64, j * 512:(j + 1) * 512],
                                                 psb[:], ACTF.Identity,
                                                 bias=biast[:])
                    if bT is not None:
                        # transposed b-tables: bT[i, c-of-tile-t] per point i
                        for t_ in range(T):
                            psb = tps.tile([128, 64], F32, tag="psbT")
                            nc.tensor.matmul(psb[:],
                                             src[:, t_ * 128:(t_ + 1) * 128],
                                             bwt[:], start=True, stop=True)
                            nc.scalar.copy(out=bT[:, t_ * 64:(t_ + 1) * 64],
                                           in_=psb[:])
                    nc.sync.dma_start(out=arep[64:128, :], in_=arep[0:64, :])
                    if bT is None:
                        nc.sync.dma_start(out=brep[64:128, 0:N - 128],
                                          in_=brep[0:64, 128:N])
                    if ctx1 is not None:
                        ctx2.__exit__(None, None, None)
                        ctx1.__exit__(None, None, None)

            def edge_round(arep, brep, wt_t, bias_t, xout, last=False,
                           pools=None):
                with nc.named_scope("round"):
                    esp, eps = pools
                    KI = KNBR * 128
                    for p in range(PAIRS):
                        tA = 2 * p
                        ga = esp.tile([128, KI], F32, tag="ga")
                        nc.gpsimd.ap_gather(ga[:], arep[:],
                                            wraps[:, p * 320:(p + 1) * 320]
                                            .bitcast(I16),
                                            channels=128, num_elems=N, d=1,
                                            num_idxs=KI)
                        gab = esp.tile([128, KI], CD, tag="gab")
                        nc.scalar.copy(out=gab[:], in_=ga[:])
                        if last:
                            # k-max via 2x fp16 TT tree in gab: 40->20->10->5
                            nc.vector.tensor_tensor(
                                out=gab[:, 0:KI // 2], in0=gab[:, 0:KI // 2],
                                in1=gab[:, KI // 2:KI], op=OP.max)
                            nc.vector.tensor_tensor(
                                out=gab[:, 0:KI // 4], in0=gab[:, 0:KI // 4],
                                in1=gab[:, KI // 4:KI // 2], op=OP.max)
                            nc.vector.tensor_tensor(
                                out=gab[:, 0:KI // 8], in0=gab[:, 0:KI // 8],
                                in1=gab[:, KI // 8:KI // 4], op=OP.max)
                            mx = esp.tile([128, 128], F32, tag="mx")
                            nc.vector.tensor_reduce(
                                out=mx[:],
                                in_=gab[:, 0:KI // 8]
                                .rearrange("p (k i) -> p i k", k=5),
                                axis=AX.X, op=OP.max)
                            zz = esp.tile([128, 128], F32, tag="zz")
                            nc.vector.tensor_tensor(
                                out=zz[:], in0=mx[:],
                                in1=brep[:, tA * 128: tA * 128 + 128], op=OP.add)
                            xo = esp.tile([128, 128], F32R, tag="xo")
                            nc.scalar.activation(xo[:], zz[:], ACTF.Prelu, alpha=LEAK)
                        else:
                            bview = brep[:, tA * 128: tA * 128 + 128] \
                                .unsqueeze(1).broadcast_to([128, KNBR, 128])
                            e = esp.tile([128, KI], CD, tag="e", bufs=4)
                            nc.vector.tensor_tensor(
                                out=e.rearrange("p (k i) -> p k i", k=KNBR),
                                in0=gab.rearrange("p (k i) -> p k i", k=KNBR),
                                in1=bview, op=OP.add)
                            nc.scalar.activation(e[:], e[:], ACTF.Prelu,
                                                 alpha=LEAK)
                            # conv matmul in 1536-col PSUM groups (12 k each)
                            GRP = [(0, 12), (1, 12), (2, 12), (3, 4)]
                            pmax = esp.tile([128, 4 * 128], F32, tag="pmax", bufs=1)
                            for g, kw in GRP:
                                cps = eps.tile([128, 1536], F32, tag="cps", bufs=2)
                                for j in range(kw // 4):
                                    c0 = g * 1536 + j * 512
                                    nc.tensor.matmul(
                                        cps[0:64, j * 512:(j + 1) * 512],
                                        wt_t[0:64, :],
                                        e[0:64, c0:c0 + 512],
                                        start=True, stop=True)
                                    nc.tensor.matmul(
                                        cps[64:128, j * 512:(j + 1) * 512],
                                        wt_t[64:128, :],
                                        e[64:128, c0:c0 + 512],
                                        start=True, stop=True)
                                nc.vector.tensor_reduce(
                                    out=pmax[:, g * 128:(g + 1) * 128],
                                    in_=cps[:, 0:kw * 128]
                                    .rearrange("p (k i) -> p i k", k=kw),
                                    axis=AX.X, op=OP.max)
                            mx = esp.tile([128, 128], F32, tag="mx")
                            nc.vector.tensor_reduce(
                                out=mx[:],
                                in_=pmax.rearrange("p (k i) -> p i k", k=4),
                                axis=AX.X, op=OP.max)
                            xo = esp.tile([128, 128], F32R, tag="xo")
                            nc.scalar.activation(xo[:], mx[:], ACTF.Prelu,
                                                 bias=bias_t[:], alpha=LEAK)
                        nc.sync.dma_start(out=xout[:, tA * 128:(tA + 1) * 128],
                                          in_=xo[0:64, :])
                        nc.sync.dma_start(out=xout[:, (tA + 1) * 128:(tA + 2) * 128],
                                          in_=xo[64:128, :])

            def load_bias128(bias_d_, pool, tag="bias128"):
                bt = pool.tile([128, 1], F32, tag=tag)
                nc.sync.dma_start(out=bt[0:64, :], in_=bias_d_[:])
                nc.sync.dma_start(out=bt[64:128, :], in_=bias_d_[:])
                return bt

            # ---- EdgeConv 1 ----
            if sel_mode == "fast2":
                scp.__exit__(None, None, None)
            xp.__exit__(None, None, None)
            b2r = load_bias128(b2_d, ecp1)
            er_sbp = tc.tile_pool(name="er_sb", bufs=2)
            er_psp = tc.tile_pool(name="er_ps", bufs=2, space="PSUM")
            erpools = (er_sbp.__enter__(), er_psp.__enter__())
            tb_sbp = tc.tile_pool(name="tb_sb", bufs=2)
            tb_psp = tc.tile_pool(name="tb_ps", bufs=1, space="PSUM")
            tbpools = (tb_sbp.__enter__(), tb_psp.__enter__())
            esp0 = erpools[0]
            a1rep = esp0.tile([128, N], F32, tag="arepX", bufs=1)
            b1rep = esp0.tile([128, N], CD, tag="brepX", bufs=1)
            with tc.tile_pool(name="xap", bufs=1) as xap:
                xa2 = xap.tile([4, N], F32R)
                nc.sync.dma_start(out=xa2[:], in_=xa_d[:])
                build_tables(a1w_d, b1w_d, b1_d, xa2[0:3, :], a1rep, b1rep, 3,
                             pools=tbpools)
            edge_round(a1rep, b1rep, w2t, b2r, x1, pools=erpools)

            # ---- EdgeConv 2 + 3 (shared pool, tag-reused tables) ----
            with tc.tile_pool(name="ec23", bufs=1) as ecp:
                a3rep = esp0.tile([128, N], F32, tag="arepX", bufs=1)
                b3rep = esp0.tile([128, N], CD, tag="brepX", bufs=1)
                w4t = ecp.tile([128, 64], CD)
                nc.sync.dma_start(out=w4t[0:64, :], in_=w4t_d[:])
                nc.sync.dma_start(out=w4t[64:128, :], in_=w4t_d[:])
                b4r = load_bias128(b4_d, ecp)
                build_tables(a3w_d, b3w_d, b3_d, x1, a3rep, b3rep, 64,
                             pools=tbpools)
                edge_round(a3rep, b3rep, w4t, b4r, x2, pools=erpools)

                a5rep = esp0.tile([128, N], F32, tag="arepX", bufs=1)
                b5rep = esp0.tile([128, N], CD, tag="brepX", bufs=1)
                build_tables(a5w_d, b5w_d, b5_d, x2, a5rep, b5rep, 64,
                             pools=tbpools)
                edge_round(a5rep, b5rep, None, None, x3, last=True,
                           pools=erpools)

            tb_psp.__exit__(None, None, None)
            tb_sbp.__exit__(None, None, None)
            er_psp.__exit__(None, None, None)
            er_sbp.__exit__(None, None, None)
            ec1p.__exit__(None, None, None)

            # =====================================================
            # Head
            # =====================================================
            with nc.named_scope("head"), \
                 tc.tile_pool(name="hd", bufs=1) as hp, \
                 tc.tile_pool(name="hd_sb", bufs=4) as hsp, \
                 tc.tile_pool(name="hd_ps", bufs=4, space="PSUM") as hps:
                w6t = hp.tile([64, 3 * 1024], F32R)
                nc.sync.dma_start(out=w6t[:], in_=w6t_d[:])
                b6t = hp.tile([128, 8], F32)
                nc.sync.dma_start(out=b6t[:], in_=b6_d[:])
                w7gt = hp.tile([128, 8 * 4 * 128], F32)
                nc.sync.dma_start(out=w7gt[:], in_=w7gt_d[:])
                b7t = hp.tile([128, 4], F32)
                nc.sync.dma_start(out=b7t[:], in_=b7_d[:])
                w7xt = hp.tile([64, 3 * 4 * 128], F32R)
                nc.sync.dma_start(out=w7xt[:], in_=w7xt_d[:])
                w8t = hp.tile([128, 4 * 2 * 128], CD)
                nc.sync.dma_start(out=w8t[:], in_=w8t_d[:])
                b8t = hp.tile([128, 2], F32)
                nc.sync.dma_start(out=b8t[:], in_=b8_d[:])
                w9t = hp.tile([128, 2 * 63], CD)
                nc.sync.dma_start(out=w9t[:], in_=w9t_d[:])
                b9t = hp.tile([63, 1], F32)
                nc.sync.dma_start(out=b9t[:], in_=b9_d[:])

                if debug:
                    nc.sync.dma_start(out=x1_dbg[:], in_=x1.bitcast(F32)[:, :])
                    nc.sync.dma_start(out=x2_dbg[:], in_=x2.bitcast(F32)[:, :])
                    nc.sync.dma_start(out=x3_dbg[:], in_=x3.bitcast(F32)[:, :])
                xs_ = [x1, x2, x3]
                NC6 = N // 512
                gtmp = hp.tile([128, 8 * NC6], F32)
                for o in range(8):
                    for n in range(NC6):
                        ps6 = hps.tile([128, 512], F32, tag="hps")
                        for kp in range(3):
                            nc.tensor.matmul(
                                ps6[:],
                                w6t[:, kp * 1024 + o * 128: kp * 1024 + (o + 1) * 128],
                                xs_[kp][:, n * 512:(n + 1) * 512],
                                start=(kp == 0), stop=(kp == 2))
                        nc.vector.tensor_reduce(
                            out=gtmp[:, o * NC6 + n: o * NC6 + n + 1],
                            in_=ps6[:], axis=AX.X, op=OP.max)
                g = hp.tile([128, 8], F32)
                nc.vector.tensor_reduce(
                    out=g[:], in_=gtmp.rearrange("p (o n) -> p o n", o=8),
                    axis=AX.X, op=OP.max)
                nc.vector.tensor_tensor(out=g[:], in0=g[:], in1=b6t[:], op=OP.add)
                g2 = hp.tile([128, 8], F32)
                nc.vector.scalar_tensor_tensor(
                    out=g2[:], in0=g[:], scalar=LEAK, in1=g[:],
                    op0=OP.mult, op1=OP.max)
                if debug:
                    nc.sync.dma_start(out=g_dbg[:], in_=g2[:])

                ps7v = hps.tile([128, 4], F32, tag="ps7v", bufs=1)
                for m in range(4):
                    for o in range(8):
                        nc.tensor.matmul(
                            ps7v[:, m:m + 1],
                            w7gt[:, (o * 4 + m) * 128:(o * 4 + m + 1) * 128],
                            g2[:, o:o + 1], start=(o == 0), stop=(o == 7))
                v7 = hp.tile([128, 4], F32)
                nc.vector.tensor_tensor(out=v7[:], in0=ps7v[:], in1=b7t[:], op=OP.add)

                for n in range(NC6):
                    y7 = hsp.tile([128, 4 * 512], CD, tag="y7", bufs=2)
                    for m in range(4):
                        ps7 = hps.tile([128, 512], F32, tag="hps")
                        for kp in range(3):
                            nc.tensor.matmul(
                                ps7[:],
                                w7xt[:, (kp * 4 + m) * 128:(kp * 4 + m + 1) * 128],
                                xs_[kp][:, n * 512:(n + 1) * 512],
                                start=(kp == 0), stop=(kp == 2))
                        nc.scalar.activation(y7[:, m * 512:(m + 1) * 512], ps7[:],
                                             ACTF.Prelu, bias=v7[:, m:m + 1],
                                             alpha=LEAK)
                    y8 = hsp.tile([128, 2 * 512], CD, tag="y8")
                    for m in range(2):
                        ps8 = hps.tile([128, 512], F32, tag="hps")
                        for k in range(4):
                            nc.tensor.matmul(
                                ps8[:], w8t[:, (k * 2 + m) * 128:(k * 2 + m + 1) * 128],
                                y7[:, k * 512:(k + 1) * 512],
                                start=(k == 0), stop=(k == 3))
                        nc.scalar.activation(y8[:, m * 512:(m + 1) * 512], ps8[:],
                                             ACTF.Prelu, bias=b8t[:, m:m + 1],
                                             alpha=LEAK)
                    ps9 = hps.tile([63, 512], F32, tag="hps")
                    for k in range(2):
                        nc.tensor.matmul(ps9[:], w9t[:, k * 63:(k + 1) * 63],
                                         y8[:, k * 512:(k + 1) * 512],
                                         start=(k == 0), stop=(k == 1))
                    o9 = hsp.tile([63, 512], F32, tag="o9")
                    nc.scalar.activation(o9[:], ps9[:], ACTF.Identity, bias=b9t[:])
                    nc.sync.dma_start(out=out_d[:, n * 512:(n + 1) * 512], in_=o9[:])

    nc.finalize()
    return nc


# =====================================================================
# Host-side folding
# =====================================================================
def fold_weights(inp):
    """inp: the reference setup_inputs() dict. Returns dict of shared
    (sample-independent) device arrays."""
    def f64(a):
        return np.asarray(a, np.float64)

    out = {}
    W1, s1, b1 = f64(inp["W1"]), f64(inp["s1"]), f64(inp["b1"])
    W1a, W1b = W1[:, :3], W1[:, 3:]
    out["a1w"] = (s1[:, None] * W1a).T.astype(np.float32).copy()
    out["b1w"] = (s1[:, None] * (W1b - W1a)).T.astype(np.float32).copy()
    out["b1"] = b1[:, None].astype(np.float32)
    W2, s2, b2 = f64(inp["W2"]), f64(inp["s2"]), f64(inp["b2"])
    assert (s2 > 0).all()
    out["w2t"] = (s2[:, None] * W2).T.astype(np.float32).copy()
    out["b2"] = b2[:, None].astype(np.float32)
    W3, s3, b3 = f64(inp["W3"]), f64(inp["s3"]), f64(inp["b3"])
    W3a, W3b = W3[:, :64], W3[:, 64:]
    out["a3w"] = (s3[:, None] * W3a).T.astype(np.float32).copy()
    out["b3w"] = (s3[:, None] * (W3b - W3a)).T.astype(np.float32).copy()
    out["b3"] = b3[:, None].astype(np.float32)
    W4, s4, b4 = f64(inp["W4"]), f64(inp["s4"]), f64(inp["b4"])
    assert (s4 > 0).all()
    out["w4t"] = (s4[:, None] * W4).T.astype(np.float32).copy()
    out["b4"] = b4[:, None].astype(np.float32)
    W5, s5, b5 = f64(inp["W5"]), f64(inp["s5"]), f64(inp["b5"])
    W5a, W5b = W5[:, :64], W5[:, 64:]
    out["a5w"] = (s5[:, None] * W5a).T.astype(np.float32).copy()
    out["b5w"] = (s5[:, None] * (W5b - W5a)).T.astype(np.float32).copy()
    out["b5"] = b5[:, None].astype(np.float32)
    W6, s6, b6 = f64(inp["W6"]), f64(inp["s6"]), f64(inp["b6"])
    assert (s6 > 0).all()
    W6f = s6[:, None] * W6
    out["w6t"] = W6f.T.reshape(3, 64, 1024).transpose(1, 0, 2) \
        .reshape(64, 3 * 1024).astype(np.float32).copy()
    out["b6"] = b6.reshape(8, 128).T.astype(np.float32).copy()
    W7, s7, b7 = f64(inp["W7"]), f64(inp["s7"]), f64(inp["b7"])
    W7f = s7[:, None] * W7
    W7g, W7x = W7f[:, :1024], W7f[:, 1024:]
    out["w7gt"] = W7g.T.reshape(8, 128, 4, 128).transpose(1, 0, 2, 3) \
        .reshape(128, -1).astype(np.float32).copy()
    out["b7"] = b7.reshape(4, 128).T.astype(np.float32).copy()
    out["w7xt"] = W7x.T.reshape(3, 64, 4, 128).transpose(1, 0, 2, 3) \
        .reshape(64, -1).astype(np.float32).copy()
    W8, s8, b8 = f64(inp["W8"]), f64(inp["s8"]), f64(inp["b8"])
    W8f = s8[:, None] * W8
    out["w8t"] = W8f.T.reshape(4, 128, 2, 128).transpose(1, 0, 2, 3) \
        .reshape(128, -1).astype(np.float32).copy()
    out["b8"] = b8.reshape(2, 128).T.astype(np.float32).copy()
    out["w9t"] = f64(inp["W9"]).T.reshape(2, 128, 63).transpose(1, 0, 2) \
        .reshape(128, 2 * 63).astype(np.float32).copy()
    out["b9"] = f64(inp["b9"])[:, None].astype(np.float32)
    return out


def fold_sample(sample_x):
    """sample_x: (3, N) float32. Returns per-sample arrays."""
    x = np.asarray(sample_x, np.float64)
    xx = (x * x).sum(0)
    N = x.shape[1]
    return {
        "xr": np.concatenate([x, -0.5 * xx[None, :]], 0).astype(np.float32),
        "xa": np.concatenate([x, np.ones((1, N))], 0).astype(np.float32),
    }


def make_in_maps(inputs, n_cores=4):
    """inputs: reference setup_inputs() dict (numpy). One core per sample."""
    shared = fold_weights(inputs)
    x = np.asarray(inputs["x"])
    in_maps = []
    for c in range(n_cores):
        b = c % x.shape[0]
        m = dict(shared)
        m.update(fold_sample(x[b]))
        in_maps.append(m)
    return in_maps


def cast_inputs(in_maps, nc):
    dts = {}
    for alloc in nc.m.functions[0].allocations:
        if isinstance(alloc, mybir.MemoryLocationSet) and alloc.kind == "ExternalInput":
            dts[alloc.memorylocations[0].name] = mybir.dt.np(alloc.dtype)
    outs = []
    for m in in_maps:
        outs.append({k: np.ascontiguousarray(np.asarray(v).astype(dts[k]))
                     for k, v in m.items() if k in dts})
    return outs


# =====================================================================
# Harness entry point
# =====================================================================
_CACHE = {}


def _make_runner(nc, n_cores):
    """Compile-once SPMD runner (mirrors bass2jax.run_bass_via_pjrt but
    caches the jitted executable across kernel() calls)."""
    import jax
    from concourse import bass2jax
    from concourse.bass2jax import _bass_exec_p, partition_id_tensor, \
        install_neuronx_cc_hook

    install_neuronx_cc_hook()
    partition_name = nc.partition_id_tensor.name if nc.partition_id_tensor else None
    in_names, out_names, out_avals, zero_shapes = [], [], [], []
    for alloc in nc.m.functions[0].allocations:
        if not isinstance(alloc, mybir.MemoryLocationSet):
            continue
        name = alloc.memorylocations[0].name
        if alloc.kind == "ExternalInput":
            if name != partition_name:
                in_names.append(name)
        elif alloc.kind == "ExternalOutput":
            shape = tuple(alloc.tensor_shape)
            dtype = mybir.dt.np(alloc.dtype)
            out_names.append(name)
            out_avals.append(jax.core.ShapedArray(shape, dtype))
            zero_shapes.append((shape, dtype))
    n_params = len(in_names)
    all_names = in_names + out_names + ([partition_name] if partition_name else [])
    donate = tuple(range(n_params, n_params + len(out_names)))

    def _body(*args):
        operands = list(args)
        if partition_name is not None:
            operands.append(partition_id_tensor())
        return tuple(_bass_exec_p.bind(
            *operands, out_avals=tuple(out_avals), in_names=tuple(all_names),
            out_names=tuple(out_names), lowering_input_output_aliases=(),
            sim_require_finite=True, sim_require_nnan=True, nc=nc))

    from jax.experimental.shard_map import shard_map
    from jax.sharding import Mesh, PartitionSpec
    mesh = Mesh(np.asarray(jax.devices()[:n_cores]), ("core",))
    in_specs = (PartitionSpec("core"),) * (n_params + len(out_names))
    out_specs = (PartitionSpec("core"),) * len(out_names)
    jf = jax.jit(
        shard_map(_body, mesh=mesh, in_specs=in_specs, out_specs=out_specs,
                  check_rep=False),
        donate_argnums=donate, keep_unused=True)

    import hashlib
    dev_cache = {}

    def _zeros_dev():
        return [jax.numpy.zeros((n_cores * shape[0],) + shape[1:], dtype)
                for shape, dtype in zero_shapes]

    def run(in_maps):
        h = hashlib.md5()
        for name in in_names:
            for m in in_maps:
                h.update(np.asarray(m[name]).tobytes())
        key = h.hexdigest()
        if key not in dev_cache:
            dev_cache.clear()
            arrs = [np.concatenate([np.asarray(m[name]) for m in in_maps], axis=0)
                    for name in in_names]
            dev_cache[key] = [jax.device_put(a) for a in arrs]
        args = list(dev_cache[key]) + _zeros_dev()
        outs = jf(*args)
        return [{n: np.asarray(outs[i]).reshape((n_cores,) + zero_shapes[i][0])[c]
                 for i, n in enumerate(out_names)}
                for c in range(n_cores)]

    return run


def kernel(**inputs):
    """DGCNN forward. inputs keyed as reference.setup_inputs(); returns
    (B, 63, N) float32. Data-parallel: one NeuronCore per sample."""
    from concourse.bass_utils import run_bass_kernel_spmd

    x = np.asarray(inputs["x"])
    B, _, N = x.shape
    key = (B, N)
    if key not in _CACHE:
        nc = build_core(N=N, conv_dtype="fp16", sel_mode="fast2")
        runner = None
        try:
            runner = _make_runner(nc, B)
        except Exception:
            runner = None
        _CACHE[key] = (nc, runner)
    nc, runner = _CACHE[key]
    in_maps = cast_inputs(make_in_maps(inputs, n_cores=B), nc)
    if runner is not None:
        try:
            results = runner(in_maps)
            return np.stack([results[b]["out"] for b in range(B)]).astype(np.float32)
        except Exception:
            _CACHE[key] = (nc, None)
    res = run_bass_kernel_spmd(nc, in_maps, core_ids=list(range(B)))
    return np.stack([res.results[b]["out"] for b in range(B)]).astype(np.float32)



# revision 26
# speedup vs baseline: 1.1170x; 1.0217x over previous
"""DGCNN Bass kernel for trn2 — per-core builder + host-side folding.

Per core (one sample, N points, k=40 neighbors):
  1. kNN scores via K=4 matmul (s_ij = x_i.x_j - 0.5|x_j|^2; row-affine
     equivalent to the reference's pairwise -dist^2).
  2. top-40 selection on DVE (max8 / max_index / match_replace rounds).
  3. EdgeConv rounds with gather-after-matmul factorization:
     conv([nbr-ctr, ctr]) = A[:, j] + B[:, i], A/B per-point tables.
  4. Global-max head with W7 split (g-part reduces to a per-channel bias).

BN scales folded into weights on host; LeakyReLU commutes with the k/N max
reductions (positive BN scale asserted host-side).
"""
import numpy as np
import concourse.bass as bass
import concourse.mybir as mybir
from concourse.bacc import Bacc
from concourse.tile import TileContext

F32 = mybir.dt.float32
F32R = mybir.dt.float32r
BF16 = mybir.dt.bfloat16
FP16 = mybir.dt.float16
U16 = mybir.dt.uint16
I16 = mybir.dt.int16
AX = mybir.AxisListType
OP = mybir.AluOpType
ACTF = mybir.ActivationFunctionType

KNBR = 40
NEG = -1e30
LEAK = 0.2


def build_core(N=4096, conv_dtype="bf16", sel_mode="mono"):
    nc = Bacc(None)
    T = N // 128
    PAIRS = T // 2
    CD = {"f32": F32, "bf16": BF16, "fp16": FP16}[conv_dtype]

    def din(name, shape, dt=F32):
        return nc.dram_tensor(name, shape, dt, kind="ExternalInput")

    xr_d = din("xr", [4, N], F32R)
    xa_d = din("xa", [4, N], F32R)
    a1w_d = din("a1w", [3, 64], F32R)
    b1w_d = din("b1w", [3, 64], F32R)
    b1_d = din("b1", [64, 1])
    w2t_d = din("w2t", [64, 64], CD)
    b2_d = din("b2", [64, 1])
    a3w_d = din("a3w", [64, 64], F32R)
    b3w_d = din("b3w", [64, 64], F32R)
    b3_d = din("b3", [64, 1])
    w4t_d = din("w4t", [64, 64], CD)
    b4_d = din("b4", [64, 1])
    a5w_d = din("a5w", [64, 64], F32R)
    b5w_d = din("b5w", [64, 64], F32R)
    b5_d = din("b5", [64, 1])
    w6t_d = din("w6t", [64, 3 * 1024], F32R)
    b6_d = din("b6", [128, 8])
    w7gt_d = din("w7gt", [128, 8 * 4 * 128])
    b7_d = din("b7", [128, 4])
    w7xt_d = din("w7xt", [64, 3 * 4 * 128], F32R)
    w8t_d = din("w8t", [128, 4 * 2 * 128], CD)
    b8_d = din("b8", [128, 2])
    w9t_d = din("w9t", [128, 2 * 63], CD)
    b9_d = din("b9", [63, 1])

    out_d = nc.dram_tensor("out", [63, N], F32, kind="ExternalOutput")
    debug = bool(int(__import__("os").environ.get("DGCNN_DEBUG", "0")))
    if debug:
        idx_dbg = nc.dram_tensor("idx_dbg", [128, T * KNBR], U16, kind="ExternalOutput")
        pooled_dbg = nc.dram_tensor("pooled_dbg", [128, T * (N // 8)], F32,
                                    kind="ExternalOutput")
        sbf_dbg = nc.dram_tensor("sbf_dbg", [128, 4 * N], FP16,
                                 kind="ExternalOutput")
        cand_dbg = nc.dram_tensor("cand_dbg", [128, 4 * 328], FP16,
                                  kind="ExternalOutput")
        nbrx_dbg = nc.dram_tensor("nbrx_dbg", [128, T * 48], U16,
                                  kind="ExternalOutput")
        x1_dbg = nc.dram_tensor("x1_dbg", [64, N], F32, kind="ExternalOutput")
        x2_dbg = nc.dram_tensor("x2_dbg", [64, N], F32, kind="ExternalOutput")
        x3_dbg = nc.dram_tensor("x3_dbg", [64, N], F32, kind="ExternalOutput")
        g_dbg = nc.dram_tensor("g_dbg", [128, 8], F32, kind="ExternalOutput")

    with TileContext(nc) as tc:
        with tc.tile_pool(name="persist", bufs=1) as pp:
            # per-pair wrapped edge lists, fully replicated: pair p at cols
            # p*320.., tile 2p in partitions 0-63 (4x16 copies), tile 2p+1
            # in partitions 64-127.
            wraps = pp.tile([128, 320 * (T // 2)], U16)
            x1 = pp.tile([64, N], F32R)
            x2 = pp.tile([64, N], F32R)
            x3 = pp.tile([64, N], F32R)
            nbr_all = (pp.tile([128, T * KNBR], U16)
                       if sel_mode != "fast2" else None)
            if sel_mode == "mono":
                pass
            elif sel_mode == "fast2":
                pass
            else:
                zc = pp.tile([128, 1], F32)
                nc.gpsimd.memset(zc[:], 0.0)
                iotaJ = pp.tile([128, N], U16)
                nc.gpsimd.iota(iotaJ[:], pattern=[[1, N]], base=0,
                               channel_multiplier=0)
                iota256f = pp.tile([128, 256], F32)
                nc.gpsimd.iota(iota256f[:], pattern=[[1, 256]], base=0,
                               channel_multiplier=0,
                               allow_small_or_imprecise_dtypes=True)
                iotaR1 = pp.tile([128, KNBR], U16)
                nc.gpsimd.iota(iotaR1[:], pattern=[[1, KNBR]], base=1,
                               channel_multiplier=0)

            # =====================================================
            # Stage A: kNN + top-40 per tile (monolithic rounds)
            # (ec1 pool + xp pool opened around it: LIFO scoping)
            # =====================================================
            ec1p = tc.tile_pool(name="ec1", bufs=1)
            ecp1 = ec1p.__enter__()
            w2t = ecp1.tile([128, 64], CD, name="w2t")
            nc.sync.dma_start(out=w2t[0:64, :], in_=w2t_d[:])
            nc.sync.dma_start(out=w2t[64:128, :], in_=w2t_d[:])
            xp = tc.tile_pool(name="xp", bufs=1)
            xpp = xp.__enter__()
            xr = xpp.tile([4, N], F32R, name="xr")
            nc.sync.dma_start(out=xr[:], in_=xr_d[:])
            xa = xpp.tile([4, N], F32R, name="xa")
            nc.sync.dma_start(out=xa[:], in_=xa_d[:])
            if sel_mode == "fast2":
                scp = tc.tile_pool(name="selconst", bufs=1)
                scpp = scp.__enter__()
                zf = scpp.tile([128, N // 8], FP16)
                nc.gpsimd.memset(zf[:], 0.0)
                iotaW = scpp.tile([128, N // 8], U16)
                nc.gpsimd.iota(iotaW[:], pattern=[[1, N // 8]], base=0,
                               channel_multiplier=0)
                iotaE40 = scpp.tile([128, 8 * KNBR], U16)
                nc.gpsimd.iota(iotaE40[:], pattern=[[1, 8], [0, KNBR]], base=0,
                               channel_multiplier=0)
            # =====================================================
            # EdgeConv machinery
            # =====================================================
            def build_tables(aw_d, bw_d, bias_d, src, arep, brep, kdim,
                             bT=None, pools=None):
                with nc.named_scope("tables"):
                    if pools is None:
                        ctx1 = tc.tile_pool(name="tb_sb", bufs=2)
                        ctx2 = tc.tile_pool(name="tb_ps", bufs=2, space="PSUM")
                        tsp, tps = ctx1.__enter__(), ctx2.__enter__()
                    else:
                        ctx1 = ctx2 = None
                        tsp, tps = pools
                    awt = tsp.tile([kdim, 64], F32R, tag="awt")
                    bwt = tsp.tile([kdim, 64], F32R, tag="bwt")
                    nc.sync.dma_start(out=awt[:], in_=aw_d[:])
                    nc.sync.dma_start(out=bwt[:], in_=bw_d[:])
                    if bT is None:
                        biast = tsp.tile([64, 1], F32, tag="biast")
                        nc.sync.dma_start(out=biast[:], in_=bias_d[:])
                    for j in range(N // 512):
                        srcj = src[:, j * 512:(j + 1) * 512]
                        psa = tps.tile([64, 512], F32, tag="psa")
                        nc.tensor.matmul(psa[:], awt[:], srcj,
                                         start=True, stop=True)
                        nc.scalar.copy(out=arep[0:64, j * 512:(j + 1) * 512], in_=psa[:])
                        if bT is None:
                            psb = tps.tile([64, 512], F32, tag="psb")
                            nc.tensor.matmul(psb[:], bwt[:], srcj,
                                             start=True, stop=True)
                            nc.scalar.activation(brep[0:64, j * 512:(j + 1) * 512],
                                                 psb[:], ACTF.Identity,
                                                 bias=biast[:])
                    if bT is not None:
                        # transposed b-tables: bT[i, c-of-tile-t] per point i
                        for t_ in range(T):
                            psb = tps.tile([128, 64], F32, tag="psbT")
                            nc.tensor.matmul(psb[:],
                                             src[:, t_ * 128:(t_ + 1) * 128],
                                             bwt[:], start=True, stop=True)
                            nc.scalar.copy(out=bT[:, t_ * 64:(t_ + 1) * 64],
                                           in_=psb[:])
                    nc.sync.dma_start(out=arep[64:128, :], in_=arep[0:64, :])
                    if bT is None:
                        nc.sync.dma_start(out=brep[64:128, 0:N - 128],
                                          in_=brep[0:64, 128:N])
                    if ctx1 is not None:
                        ctx2.__exit__(None, None, None)
                        ctx1.__exit__(None, None, None)

            def edge_round(arep, brep, wt_t, bias_t, xout, last=False,
                           pools=None):
                with nc.named_scope("round"):
                    esp, eps = pools
                    KI = KNBR * 128
                    for p in range(PAIRS):
                        tA = 2 * p
                        ga = esp.tile([128, KI], F32, tag="ga")
                        nc.gpsimd.ap_gather(ga[:], arep[:],
                                            wraps[:, p * 320:(p + 1) * 320]
                                            .bitcast(I16),
                                            channels=128, num_elems=N, d=1,
                                            num_idxs=KI)
                        gab = esp.tile([128, KI], CD, tag="gab")
                        nc.scalar.copy(out=gab[:], in_=ga[:])
                        if last:
                            # k-max via 2x fp16 TT tree in gab: 40->20->10->5
                            nc.vector.tensor_tensor(
                                out=gab[:, 0:KI // 2], in0=gab[:, 0:KI // 2],
                                in1=gab[:, KI // 2:KI], op=OP.max)
                            nc.vector.tensor_tensor(
                                out=gab[:, 0:KI // 4], in0=gab[:, 0:KI // 4],
                                in1=gab[:, KI // 4:KI // 2], op=OP.max)
                            nc.vector.tensor_tensor(
                                out=gab[:, 0:KI // 8], in0=gab[:, 0:KI // 8],
                                in1=gab[:, KI // 8:KI // 4], op=OP.max)
                            mx = esp.tile([128, 128], F32, tag="mx")
                            nc.vector.tensor_reduce(
                                out=mx[:],
                                in_=gab[:, 0:KI // 8]
                                .rearrange("p (k i) -> p i k", k=5),
                                axis=AX.X, op=OP.max)
                            zz = esp.tile([128, 128], F32, tag="zz")
                            nc.vector.tensor_tensor(
                                out=zz[:], in0=mx[:],
                                in1=brep[:, tA * 128: tA * 128 + 128], op=OP.add)
                            xo = esp.tile([128, 128], F32R, tag="xo")
                            nc.scalar.activation(xo[:], zz[:], ACTF.Prelu, alpha=LEAK)
                        else:
                            bview = brep[:, tA * 128: tA * 128 + 128] \
                                .unsqueeze(1).broadcast_to([128, KNBR, 128])
                            e = esp.tile([128, KI], CD, tag="e", bufs=4)
                            nc.vector.tensor_tensor(
                                out=e.rearrange("p (k i) -> p k i", k=KNBR),
                                in0=gab.rearrange("p (k i) -> p k i", k=KNBR),
                                in1=bview, op=OP.add)
                            nc.scalar.activation(e[:], e[:], ACTF.Prelu,
                                                 alpha=LEAK)
                            # conv matmul in 1536-col PSUM groups (12 k each)
                            GRP = [(0, 12), (1, 12), (2, 12), (3, 4)]
                            pmax = esp.tile([128, 4 * 128], F32, tag="pmax", bufs=2)
                            for g, kw in GRP:
                                cps = eps.tile([128, 1536], F32, tag="cps", bufs=2)
                                for j in range(kw // 4):
                                    c0 = g * 1536 + j * 512
                                    nc.tensor.matmul(
                                        cps[0:64, j * 512:(j + 1) * 512],
                                        wt_t[0:64, :],
                                        e[0:64, c0:c0 + 512],
                                        start=True, stop=True)
                                    nc.tensor.matmul(
                                        cps[64:128, j * 512:(j + 1) * 512],
                                        wt_t[64:128, :],
                                        e[64:128, c0:c0 + 512],
                                        start=True, stop=True)
                                nc.vector.tensor_reduce(
                                    out=pmax[:, g * 128:(g + 1) * 128],
                                    in_=cps[:, 0:kw * 128]
                                    .rearrange("p (k i) -> p i k", k=kw),
                                    axis=AX.X, op=OP.max)
                            mx = esp.tile([128, 128], F32, tag="mx")
                            nc.vector.tensor_reduce(
                                out=mx[:],
                                in_=pmax.rearrange("p (k i) -> p i k", k=4),
                                axis=AX.X, op=OP.max)
                            xo = esp.tile([128, 128], F32R, tag="xo")
                            nc.scalar.activation(xo[:], mx[:], ACTF.Prelu,
                                                 bias=bias_t[:], alpha=LEAK)
                        nc.sync.dma_start(out=xout[:, tA * 128:(tA + 1) * 128],
                                          in_=xo[0:64, :])
                        nc.sync.dma_start(out=xout[:, (tA + 1) * 128:(tA + 2) * 128],
                                          in_=xo[64:128, :])

            def load_bias128(bias_d_, pool, tag="bias128"):
                bt = pool.tile([128, 1], F32, tag=tag)
                nc.sync.dma_start(out=bt[0:64, :], in_=bias_d_[:])
                nc.sync.dma_start(out=bt[64:128, :], in_=bias_d_[:])
                return bt

            b2r = load_bias128(b2_d, ecp1)
            er_sbp = tc.tile_pool(name="er_sb", bufs=2)
            er_psp = tc.tile_pool(name="er_ps", bufs=2, space="PSUM")
            erpools = (er_sbp.__enter__(), er_psp.__enter__())
            tb_sbp = tc.tile_pool(name="tb_sb", bufs=2)
            tb_psp = tc.tile_pool(name="tb_ps", bufs=1, space="PSUM")
            tbpools = (tb_sbp.__enter__(), tb_psp.__enter__())
            esp0 = erpools[0]
            a1rep = esp0.tile([128, N], F32, tag="arepX", bufs=1)
            b1rep = esp0.tile([128, N], CD, tag="brepX", bufs=1)
            if sel_mode == "fast2":
                build_tables(a1w_d, b1w_d, b1_d, xa[0:3, :], a1rep, b1rep, 3,
                             pools=tbpools)
            with tc.tile_pool(name="sel_sb", bufs=2) as sp, \
                 tc.tile_pool(name="sel_ps", bufs=2, space="PSUM") as sps:
                W = N // 8

                def sel_stage2(tp, cand_p, orig_p):
                    """Deferred per-tile tail: top-40 of 320 cands + wraps."""
                    with nc.named_scope("sel"):
                        m8c = sp.tile([128, 8], FP16, tag="m8c")
                        cv = cand_p[:, 0:8 * KNBR]
                        for r in range(5):
                            nc.vector.max(out=m8c[:], in_=cv)
                            nc.vector.match_replace(out=cv, in_to_replace=m8c[:],
                                                    in_values=cv, imm_value=NEG)
                        cmask = sp.tile([128, 8 * KNBR], FP16, tag="cmask")
                        nc.vector.tensor_scalar(cmask[:], cv, -1e29, None,
                                                op0=OP.is_le)
                        crank = sp.tile([128, 8 * KNBR], FP16, tag="crank")
                        nc.vector.tensor_tensor_scan(
                            crank[:], cmask[:], zf[:, 0:8 * KNBR],
                            0.0, op0=OP.add, op1=OP.add)
                        cmm1 = sp.tile([128, 8 * KNBR], FP16, tag="cmm1")
                        nc.vector.tensor_scalar(cmm1[:], cmask[:], -1.0, None,
                                                op0=OP.add)
                        cm0 = sp.tile([128, 8 * KNBR], FP16, tag="cm0")
                        nc.vector.tensor_tensor(
                            out=cm0[:], in0=crank[:], in1=cmask[:], op=OP.mult)
                        cslot = sp.tile([128, 8 * KNBR], I16, tag="cslot")
                        nc.vector.tensor_tensor(out=cslot[:], in0=cm0[:],
                                                in1=cmm1[:], op=OP.add)
                        nbrx = sp.tile([128, 48], U16, tag="nbrx", bufs=3)
                        nc.gpsimd.local_scatter(nbrx[:], orig_p[:], cslot[:],
                                                channels=128, num_elems=48,
                                                num_idxs=8 * KNBR)
                        if debug:
                            nc.sync.dma_start(
                                out=nbrx_dbg[:, tp * 48:(tp + 1) * 48],
                                in_=nbrx[:])
                    pbase = 64 * (tp % 2)
                    cb = (tp // 2) * 320
                    for ihi in range(8):
                        dst = wraps[pbase:pbase + 16, cb:cb + 320] \
                            .rearrange("p (k e) -> p k e", e=8)[:, :, ihi:ihi + 1]
                        nsrc = nbrx[ihi * 16:(ihi + 1) * 16, 1:41]
                        nc.sync.dma_start(out=dst, in_=nsrc.unsqueeze(2))
                    for rep in range(1, 4):
                        nc.sync.dma_start(
                            out=wraps[pbase + 16 * rep: pbase + 16 * (rep + 1),
                                      cb:cb + 320],
                            in_=wraps[pbase: pbase + 16, cb:cb + 320])

                pend = None
                for t in range(T):
                    if sel_mode == "fast2":
                        # e-major fp16 score plane; window maxima via 4x STT tree
                        s_sb = sp.tile([128, N], FP16, tag="s_sb", bufs=2)
                        t1 = sp.tile([128, N // 2], FP16, tag="t1", bufs=2)
                        t2 = sp.tile([128, N // 4], FP16, tag="t2", bufs=2)
                        pooled = sp.tile([128, W], FP16, tag="pooled", bufs=2)
                    else:
                        s_sb = sp.tile([128, N], F32, tag="s_sb", bufs=1)
                        if sel_mode != "mono":
                            pooled = sp.tile([128, N // 8], F32, tag="pooled",
                                             bufs=1)
                            sbf = sp.tile([128, N], BF16, tag="sbf", bufs=1)
                    with nc.named_scope("knn_score"):
                        for h in range(2):
                            ps = sps.tile([128, N // 2], F32, tag="score",
                                          bufs=1 if sel_mode == "fast2" else 2)
                            for j in range(N // 2 // 512):
                                col = h * (N // 2) + j * 512
                                nc.tensor.matmul(
                                    ps[:, j * 512:(j + 1) * 512],
                                    xa[:, t * 128:(t + 1) * 128],
                                    xr[:, col:col + 512],
                                    start=True, stop=True)
                            if sel_mode == "fast2":
                                nc.scalar.copy(
                                    out=s_sb.rearrange("p (e w) -> p e w", e=8)
                                    [:, :, h * (W // 2):(h + 1) * (W // 2)],
                                    in_=ps.rearrange("p (w e) -> p e w", e=8))
                            else:
                                nc.scalar.copy(
                                    out=s_sb[:, h * (N // 2):(h + 1) * (N // 2)],
                                    in_=ps[:])
                            if sel_mode not in ("mono", "fast2"):
                                nc.vector.tensor_reduce(
                                    out=pooled[:, h * (W // 2):(h + 1) * (W // 2)],
                                    in_=ps.rearrange("p (w k) -> p w k", k=8),
                                    axis=AX.X, op=OP.max)
                            if sel_mode == "fast2":
                                # per-half window-max tree over the 8 e-planes
                                hw = slice(h * (W // 2), (h + 1) * (W // 2))
                                sv = s_sb.rearrange("p (e w) -> p e w", e=8)
                                t1v = t1.rearrange("p (e w) -> p e w", e=4)
                                t2v = t2.rearrange("p (e w) -> p e w", e=2)
                                nc.vector.tensor_tensor(
                                    out=t1v[:, :, hw], in0=sv[:, 0:4, hw],
                                    in1=sv[:, 4:8, hw], op=OP.max)
                                nc.vector.tensor_tensor(
                                    out=t2v[:, :, hw], in0=t1v[:, 0:2, hw],
                                    in1=t1v[:, 2:4, hw], op=OP.max)
                                nc.vector.tensor_tensor(
                                    out=pooled[:, hw], in0=t2v[:, 0, hw],
                                    in1=t2v[:, 1, hw], op=OP.max)
                    sel_scope = nc.named_scope("sel")
                    sel_scope.__enter__()
                    m8 = sp.tile([128, 8], FP16 if sel_mode == "fast2" else F32,
                                 tag="m8")
                    if sel_mode == "mono":
                        for r in range(5):
                            nc.vector.max(out=m8[:], in_=s_sb[:])
                            nc.vector.max_index(
                                out=nbr_all[:, t * KNBR + r * 8: t * KNBR + (r + 1) * 8],
                                in_max=m8[:], in_values=s_sb[:])
                            nc.vector.match_replace(out=s_sb[:], in_to_replace=m8[:],
                                                    in_values=s_sb[:], imm_value=NEG)
                    elif sel_mode == "fast2":
                        # --- extract top-40 windows (exactly 40, 8 elems each)
                        for r in range(5):
                            nc.vector.max(out=m8[:], in_=pooled[:])
                            nc.vector.match_replace(
                                out=pooled[:], in_to_replace=m8[:],
                                in_values=pooled[:], imm_value=NEG)
                        wmask = sp.tile([128, W], FP16, tag="wmask")
                        nc.vector.tensor_scalar(wmask[:], pooled[:], -1e29, None,
                                                op0=OP.is_le)
                        wrank = sp.tile([128, W], FP16, tag="wrank")
                        nc.vector.tensor_tensor_scan(
                            wrank[:], wmask[:], zf[:, 0:W], 0.0,
                            op0=OP.add, op1=OP.add)
                        wmm1 = sp.tile([128, W], FP16, tag="wmm1")
                        nc.vector.tensor_scalar(wmm1[:], wmask[:], -1.0, None,
                                                op0=OP.add)
                        wm0 = sp.tile([128, W], FP16, tag="wm0")
                        nc.vector.tensor_tensor(
                            out=wm0[:], in0=wrank[:], in1=wmask[:], op=OP.mult)
                        # wm1: rank 1..40 at extracted windows, -1 elsewhere
                        wm1 = sp.tile([128, W], I16, tag="wm1")
                        nc.vector.tensor_tensor(out=wm1[:], in0=wm0[:],
                                                in1=wmm1[:], op=OP.add)
                        # window id of each rank (slot r holds window index)
                        winv = sp.tile([128, 48], U16, tag="winv", bufs=3)
                        nc.gpsimd.local_scatter(winv[:], iotaW[:], wm1[:],
                                                channels=128, num_elems=48,
                                                num_idxs=W)
                        # gather the 40 windows' contents: 8 disjoint-slice
                        # scatters sharing the window-rank index wm1, then an
                        # Act compaction to a contiguous candidate array.
                        cand8 = sp.tile([128, 8 * 42], FP16, tag="cand8")
                        for e in range(8):
                            nc.gpsimd.local_scatter(
                                cand8[:, e * 42:(e + 1) * 42],
                                s_sb[:, e * W:(e + 1) * W], wm1[:],
                                channels=128, num_elems=42, num_idxs=W)
                        cand = sp.tile([128, 8 * KNBR], FP16, tag="cand")
                        nc.scalar.copy(
                            out=cand.rearrange("p (e r) -> p e r", e=8),
                            in_=cand8.rearrange("p (e r) -> p e r", r=42)
                            [:, :, 1:41])
                        # original j of each cand slot: winv[r]*8 + e
                        winv8 = sp.tile([128, 41], U16, tag="winv8")
                        nc.vector.tensor_scalar(winv8[:], winv[:, 0:41], 8, None,
                                                op0=OP.mult)
                        orig = sp.tile([128, 8 * KNBR], U16, tag="orig")
                        nc.vector.tensor_tensor(
                            out=orig.rearrange("p (e r) -> p e r", e=8),
                            in0=winv8[:, 1:41].unsqueeze(1)
                            .broadcast_to([128, 8, KNBR]),
                            in1=iotaE40.rearrange("p (e r) -> p e r", e=8),
                            op=OP.add)
                        if debug and t < 4:
                            nc.sync.dma_start(
                                out=sbf_dbg[:, t * N:(t + 1) * N],
                                in_=s_sb[:])
                            nc.sync.dma_start(
                                out=cand_dbg[:, t * 328:(t + 1) * 328],
                                in_=cand[:])
                    else:
                        # screen for tau = 40th-largest pooled window max
                        for r in range(5):
                            nc.vector.max(out=m8[:], in_=pooled[:])
                            nc.vector.match_replace(out=pooled[:], in_to_replace=m8[:],
                                                    in_values=pooled[:], imm_value=NEG)
                        tau = m8[:, 7:8]
                        negtau = sp.tile([128, 1], F32, tag="negtau")
                        nc.vector.tensor_scalar(negtau[:], tau, -1.0, None,
                                                op0=OP.mult)
                        nc.scalar.activation(sbf[:], s_sb[:], ACTF.Identity,
                                             bias=negtau[:])
                        maskf = sp.tile([128, N], BF16, tag="maskf", bufs=1)
                        nc.vector.tensor_scalar(maskf[:], s_sb[:], tau, None,
                                                op0=OP.is_ge)
                        rankf = sp.tile([128, N], F32, tag="rankf", bufs=1)
                        nc.vector.tensor_tensor_scan(
                            rankf[:], maskf[:], zc.broadcast_to([128, N]), 0.0,
                            op0=OP.add, op1=OP.add)
                        slotf = sp.tile([128, N], I16, tag="slotf", bufs=1)
                        nc.vector.scalar_tensor_tensor(
                            out=slotf[:], in0=rankf[:], scalar=1.0, in1=maskf[:],
                            op0=OP.mult, op1=OP.mult)
                        candb = sp.tile([128, 256], BF16, tag="candb", bufs=1)
                        nc.gpsimd.local_scatter(candb[:], sbf[:], slotf[:],
                                                channels=128, num_elems=256,
                                                num_idxs=N)
                        cidx = sp.tile([128, 256], U16, tag="cidx", bufs=1)
                        nc.gpsimd.local_scatter(cidx[:], iotaJ[:], slotf[:],
                                                channels=128, num_elems=256,
                                                num_idxs=N)
                        count = rankf[:, N - 1: N]
                        emptym = sp.tile([128, 256], F32, tag="emptym", bufs=1)
                        nc.vector.tensor_scalar(emptym[:], iota256f[:], count, None,
                                                op0=OP.is_gt)
                        candfix = sp.tile([128, 256], F32, tag="candfix", bufs=1)
                        nc.vector.scalar_tensor_tensor(
                            out=candfix[:], in0=emptym[:], scalar=NEG, in1=candb[:],
                            op0=OP.mult, op1=OP.add)
                        pos40 = sp.tile([128, KNBR], U16, tag="pos40")
                        cv = candfix[:, 1:256]
                        for r in range(5):
                            nc.vector.max(out=m8[:], in_=cv)
                            nc.vector.max_index(out=pos40[:, r * 8:(r + 1) * 8],
                                                in_max=m8[:], in_values=cv)
                            nc.vector.match_replace(out=cv, in_to_replace=m8[:],
                                                    in_values=cv, imm_value=NEG)
                        posi = sp.tile([128, KNBR], I16, tag="posi")
                        nc.vector.tensor_scalar(posi[:], pos40[:], 1, None, op0=OP.add)
                        winv = sp.tile([128, 256], U16, tag="winv", bufs=1)
                        nc.gpsimd.local_scatter(winv[:], iotaR1[:], posi[:],
                                                channels=128, num_elems=256,
                                                num_idxs=KNBR)
                        winm1 = sp.tile([128, 256], I16, tag="winm1", bufs=1)
                        nc.vector.tensor_scalar(winm1[:], winv[:], 1, None,
                                                op0=OP.subtract)
                        nc.gpsimd.local_scatter(
                            nbr_all[:, t * KNBR:(t + 1) * KNBR], cidx[:], winm1[:],
                            channels=128, num_elems=KNBR, num_idxs=256)
                    sel_scope.__exit__(None, None, None)
                    if sel_mode == "fast2":
                        # two-stage pipeline: tail of tile t-1 after screen t
                        if pend is not None:
                            sel_stage2(*pend)
                        pend = (t, cand, orig)
                        continue
                    # wrapped list: wrap[ilo, kk*8+ihi] = nbr[ihi*16+ilo, kk]
                    pbase = 64 * (t % 2)
                    cb = (t // 2) * 320
                    for ihi in range(8):
                        dst = wraps[pbase:pbase + 16, cb:cb + 320] \
                            .rearrange("p (k e) -> p k e", e=8)[:, :, ihi:ihi + 1]
                        nsrc = nbr_all[ihi * 16:(ihi + 1) * 16,
                                       t * KNBR:(t + 1) * KNBR]
                        nc.sync.dma_start(out=dst, in_=nsrc.unsqueeze(2))
                    # replicate to the other three 16-partition groups
                    for rep in range(1, 4):
                        nc.sync.dma_start(
                            out=wraps[pbase + 16 * rep: pbase + 16 * (rep + 1),
                                      cb:cb + 320],
                            in_=wraps[pbase: pbase + 16, cb:cb + 320])
                if sel_mode == "fast2" and pend is not None:
                    sel_stage2(*pend)
            if debug and sel_mode != "fast2":
                nc.sync.dma_start(out=idx_dbg[:], in_=nbr_all[:])

            # ---- EdgeConv 1 (tables pre-built during selection) ----
            if sel_mode == "fast2":
                edge_round(a1rep, b1rep, w2t, b2r, x1, pools=erpools)
            else:
                with tc.tile_pool(name="xap", bufs=1) as xap:
                    xa2 = xap.tile([4, N], F32R)
                    nc.sync.dma_start(out=xa2[:], in_=xa_d[:])
                    build_tables(a1w_d, b1w_d, b1_d, xa2[0:3, :], a1rep, b1rep,
                                 3, pools=tbpools)
                edge_round(a1rep, b1rep, w2t, b2r, x1, pools=erpools)

            # ---- EdgeConv 2 + 3 (shared pool, tag-reused tables) ----
            with tc.tile_pool(name="ec23", bufs=1) as ecp:
                a3rep = esp0.tile([128, N], F32, tag="arepX", bufs=1)
                b3rep = esp0.tile([128, N], CD, tag="brepX", bufs=1)
                w4t = ecp.tile([128, 64], CD)
                nc.sync.dma_start(out=w4t[0:64, :], in_=w4t_d[:])
                nc.sync.dma_start(out=w4t[64:128, :], in_=w4t_d[:])
                b4r = load_bias128(b4_d, ecp)
                build_tables(a3w_d, b3w_d, b3_d, x1, a3rep, b3rep, 64,
                             pools=tbpools)
                edge_round(a3rep, b3rep, w4t, b4r, x2, pools=erpools)

                a5rep = esp0.tile([128, N], F32, tag="arepX", bufs=1)
                b5rep = esp0.tile([128, N], CD, tag="brepX", bufs=1)
                build_tables(a5w_d, b5w_d, b5_d, x2, a5rep, b5rep, 64,
                             pools=tbpools)
                edge_round(a5rep, b5rep, None, None, x3, last=True,
                           pools=erpools)

            tb_psp.__exit__(None, None, None)
            tb_sbp.__exit__(None, None, None)
            er_psp.__exit__(None, None, None)
            er_sbp.__exit__(None, None, None)
            if sel_mode == "fast2":
                scp.__exit__(None, None, None)
            xp.__exit__(None, None, None)
            ec1p.__exit__(None, None, None)

            # =====================================================
            # Head
            # =====================================================
            with nc.named_scope("head"), \
                 tc.tile_pool(name="hd", bufs=1) as hp, \
                 tc.tile_pool(name="hd_sb", bufs=4) as hsp, \
                 tc.tile_pool(name="hd_ps", bufs=4, space="PSUM") as hps:
                w6t = hp.tile([64, 3 * 1024], F32R)
                nc.sync.dma_start(out=w6t[:], in_=w6t_d[:])
                b6t = hp.tile([128, 8], F32)
                nc.sync.dma_start(out=b6t[:], in_=b6_d[:])
                w7gt = hp.tile([128, 8 * 4 * 128], F32)
                nc.sync.dma_start(out=w7gt[:], in_=w7gt_d[:])
                b7t = hp.tile([128, 4], F32)
                nc.sync.dma_start(out=b7t[:], in_=b7_d[:])
                w7xt = hp.tile([64, 3 * 4 * 128], F32R)
                nc.sync.dma_start(out=w7xt[:], in_=w7xt_d[:])
                w8t = hp.tile([128, 4 * 2 * 128], CD)
                nc.sync.dma_start(out=w8t[:], in_=w8t_d[:])
                b8t = hp.tile([128, 2], F32)
                nc.sync.dma_start(out=b8t[:], in_=b8_d[:])
                w9t = hp.tile([128, 2 * 63], CD)
                nc.sync.dma_start(out=w9t[:], in_=w9t_d[:])
                b9t = hp.tile([63, 1], F32)
                nc.sync.dma_start(out=b9t[:], in_=b9_d[:])

                if debug:
                    nc.sync.dma_start(out=x1_dbg[:], in_=x1.bitcast(F32)[:, :])
                    nc.sync.dma_start(out=x2_dbg[:], in_=x2.bitcast(F32)[:, :])
                    nc.sync.dma_start(out=x3_dbg[:], in_=x3.bitcast(F32)[:, :])
                xs_ = [x1, x2, x3]
                NC6 = N // 512
                gtmp = hp.tile([128, 8 * NC6], F32)
                for o in range(8):
                    for n in range(NC6):
                        ps6 = hps.tile([128, 512], F32, tag="hps")
                        for kp in range(3):
                            nc.tensor.matmul(
                                ps6[:],
                                w6t[:, kp * 1024 + o * 128: kp * 1024 + (o + 1) * 128],
                                xs_[kp][:, n * 512:(n + 1) * 512],
                                start=(kp == 0), stop=(kp == 2))
                        nc.vector.tensor_reduce(
                            out=gtmp[:, o * NC6 + n: o * NC6 + n + 1],
                            in_=ps6[:], axis=AX.X, op=OP.max)
                g = hp.tile([128, 8], F32)
                nc.vector.tensor_reduce(
                    out=g[:], in_=gtmp.rearrange("p (o n) -> p o n", o=8),
                    axis=AX.X, op=OP.max)
                nc.vector.tensor_tensor(out=g[:], in0=g[:], in1=b6t[:], op=OP.add)
                g2 = hp.tile([128, 8], F32)
                nc.vector.scalar_tensor_tensor(
                    out=g2[:], in0=g[:], scalar=LEAK, in1=g[:],
                    op0=OP.mult, op1=OP.max)
                if debug:
                    nc.sync.dma_start(out=g_dbg[:], in_=g2[:])

                ps7v = hps.tile([128, 4], F32, tag="ps7v", bufs=1)
                for m in range(4):
                    for o in range(8):
                        nc.tensor.matmul(
                            ps7v[:, m:m + 1],
                            w7gt[:, (o * 4 + m) * 128:(o * 4 + m + 1) * 128],
                            g2[:, o:o + 1], start=(o == 0), stop=(o == 7))
                v7 = hp.tile([128, 4], F32)
                nc.vector.tensor_tensor(out=v7[:], in0=ps7v[:], in1=b7t[:], op=OP.add)

                for n in range(NC6):
                    y7 = hsp.tile([128, 4 * 512], CD, tag="y7", bufs=2)
                    for m in range(4):
                        ps7 = hps.tile([128, 512], F32, tag="hps")
                        for kp in range(3):
                            nc.tensor.matmul(
                                ps7[:],
                                w7xt[:, (kp * 4 + m) * 128:(kp * 4 + m + 1) * 128],
                                xs_[kp][:, n * 512:(n + 1) * 512],
                                start=(kp == 0), stop=(kp == 2))
                        nc.scalar.activation(y7[:, m * 512:(m + 1) * 512], ps7[:],
                                             ACTF.Prelu, bias=v7[:, m:m + 1],
                                             alpha=LEAK)
                    y8 = hsp.tile([128, 2 * 512], CD, tag="y8")
                    for m in range(2):
                        ps8 = hps.tile([128, 512], F32, tag="hps")
                        for k in range(4):
                            nc.tensor.matmul(
                                ps8[:], w8t[:, (k * 2 + m) * 128:(k * 2 + m + 1) * 128],
                                y7[:, k * 512:(k + 1) * 512],
                                start=(k == 0), stop=(k == 3))
                        nc.scalar.activation(y8[:, m * 512:(m + 1) * 512], ps8[:],
                                             ACTF.Prelu, bias=b8t[:, m:m + 1],
                                             alpha=LEAK)
                    ps9 = hps.tile([63, 512], F32, tag="hps")
                    for k in range(2):
                        nc.tensor.matmul(ps9[:], w9t[:, k * 63:(k + 1) * 63],
                                         y8[:, k * 512:(k + 1) * 512],
                                         start=(k == 0), stop=(k == 1))
                    o9 = hsp.tile([63, 512], F32, tag="o9")
                    nc.scalar.activation(o9[:], ps9[:], ACTF.Identity, bias=b9t[:])
                    nc.sync.dma_start(out=out_d[:, n * 512:(n + 1) * 512], in_=o9[:])

    nc.finalize()
    return nc


# =====================================================================
# Host-side folding
# =====================================================================
def fold_weights(inp):
    """inp: the reference setup_inputs() dict. Returns dict of shared
    (sample-independent) device arrays."""
    def f64(a):
        return np.asarray(a, np.float64)

    out = {}
    W1, s1, b1 = f64(inp["W1"]), f64(inp["s1"]), f64(inp["b1"])
    W1a, W1b = W1[:, :3], W1[:, 3:]
    out["a1w"] = (s1[:, None] * W1a).T.astype(np.float32).copy()
    out["b1w"] = (s1[:, None] * (W1b - W1a)).T.astype(np.float32).copy()
    out["b1"] = b1[:, None].astype(np.float32)
    W2, s2, b2 = f64(inp["W2"]), f64(inp["s2"]), f64(inp["b2"])
    assert (s2 > 0).all()
    out["w2t"] = (s2[:, None] * W2).T.astype(np.float32).copy()
    out["b2"] = b2[:, None].astype(np.float32)
    W3, s3, b3 = f64(inp["W3"]), f64(inp["s3"]), f64(inp["b3"])
    W3a, W3b = W3[:, :64], W3[:, 64:]
    out["a3w"] = (s3[:, None] * W3a).T.astype(np.float32).copy()
    out["b3w"] = (s3[:, None] * (W3b - W3a)).T.astype(np.float32).copy()
    out["b3"] = b3[:, None].astype(np.float32)
    W4, s4, b4 = f64(inp["W4"]), f64(inp["s4"]), f64(inp["b4"])
    assert (s4 > 0).all()
    out["w4t"] = (s4[:, None] * W4).T.astype(np.float32).copy()
    out["b4"] = b4[:, None].astype(np.float32)
    W5, s5, b5 = f64(inp["W5"]), f64(inp["s5"]), f64(inp["b5"])
    W5a, W5b = W5[:, :64], W5[:, 64:]
    out["a5w"] = (s5[:, None] * W5a).T.astype(np.float32).copy()
    out["b5w"] = (s5[:, None] * (W5b - W5a)).T.astype(np.float32).copy()
    out["b5"] = b5[:, None].astype(np.float32)
    W6, s6, b6 = f64(inp["W6"]), f64(inp["s6"]), f64(inp["b6"])
    assert (s6 > 0).all()
    W6f = s6[:, None] * W6
    out["w6t"] = W6f.T.reshape(3, 64, 1024).transpose(1, 0, 2) \
        .reshape(64, 3 * 1024).astype(np.float32).copy()
    out["b6"] = b6.reshape(8, 128).T.astype(np.float32).copy()
    W7, s7, b7 = f64(inp["W7"]), f64(inp["s7"]), f64(inp["b7"])
    W7f = s7[:, None] * W7
    W7g, W7x = W7f[:, :1024], W7f[:, 1024:]
    out["w7gt"] = W7g.T.reshape(8, 128, 4, 128).transpose(1, 0, 2, 3) \
        .reshape(128, -1).astype(np.float32).copy()
    out["b7"] = b7.reshape(4, 128).T.astype(np.float32).copy()
    out["w7xt"] = W7x.T.reshape(3, 64, 4, 128).transpose(1, 0, 2, 3) \
        .reshape(64, -1).astype(np.float32).copy()
    W8, s8, b8 = f64(inp["W8"]), f64(inp["s8"]), f64(inp["b8"])
    W8f = s8[:, None] * W8
    out["w8t"] = W8f.T.reshape(4, 128, 2, 128).transpose(1, 0, 2, 3) \
        .reshape(128, -1).astype(np.float32).copy()
    out["b8"] = b8.reshape(2, 128).T.astype(np.float32).copy()
    out["w9t"] = f64(inp["W9"]).T.reshape(2, 128, 63).transpose(1, 0, 2) \
        .reshape(128, 2 * 63).astype(np.float32).copy()
    out["b9"] = f64(inp["b9"])[:, None].astype(np.float32)
    return out


def fold_sample(sample_x):
    """sample_x: (3, N) float32. Returns per-sample arrays."""
    x = np.asarray(sample_x, np.float64)
    xx = (x * x).sum(0)
    N = x.shape[1]
    return {
        "xr": np.concatenate([x, -0.5 * xx[None, :]], 0).astype(np.float32),
        "xa": np.concatenate([x, np.ones((1, N))], 0).astype(np.float32),
    }


def make_in_maps(inputs, n_cores=4):
    """inputs: reference setup_inputs() dict (numpy). One core per sample."""
    shared = fold_weights(inputs)
    x = np.asarray(inputs["x"])
    in_maps = []
    for c in range(n_cores):
        b = c % x.shape[0]
        m = dict(shared)
        m.update(fold_sample(x[b]))
        in_maps.append(m)
    return in_maps


def cast_inputs(in_maps, nc):
    dts = {}
    for alloc in nc.m.functions[0].allocations:
        if isinstance(alloc, mybir.MemoryLocationSet) and alloc.kind == "ExternalInput":
            dts[alloc.memorylocations[0].name] = mybir.dt.np(alloc.dtype)
    outs = []
    for m in in_maps:
        outs.append({k: np.ascontiguousarray(np.asarray(v).astype(dts[k]))
                     for k, v in m.items() if k in dts})
    return outs


# =====================================================================
# Harness entry point
# =====================================================================
_CACHE = {}


def _make_runner(nc, n_cores):
    """Compile-once SPMD runner (mirrors bass2jax.run_bass_via_pjrt but
    caches the jitted executable across kernel() calls)."""
    import jax
    from concourse import bass2jax
    from concourse.bass2jax import _bass_exec_p, partition_id_tensor, \
        install_neuronx_cc_hook

    install_neuronx_cc_hook()
    partition_name = nc.partition_id_tensor.name if nc.partition_id_tensor else None
    in_names, out_names, out_avals, zero_shapes = [], [], [], []
    for alloc in nc.m.functions[0].allocations:
        if not isinstance(alloc, mybir.MemoryLocationSet):
            continue
        name = alloc.memorylocations[0].name
        if alloc.kind == "ExternalInput":
            if name != partition_name:
                in_names.append(name)
        elif alloc.kind == "ExternalOutput":
            shape = tuple(alloc.tensor_shape)
            dtype = mybir.dt.np(alloc.dtype)
            out_names.append(name)
            out_avals.append(jax.core.ShapedArray(shape, dtype))
            zero_shapes.append((shape, dtype))
    n_params = len(in_names)
    all_names = in_names + out_names + ([partition_name] if partition_name else [])
    donate = tuple(range(n_params, n_params + len(out_names)))

    def _body(*args):
        operands = list(args)
        if partition_name is not None:
            operands.append(partition_id_tensor())
        return tuple(_bass_exec_p.bind(
            *operands, out_avals=tuple(out_avals), in_names=tuple(all_names),
            out_names=tuple(out_names), lowering_input_output_aliases=(),
            sim_require_finite=True, sim_require_nnan=True, nc=nc))

    from jax.experimental.shard_map import shard_map
    from jax.sharding import Mesh, PartitionSpec
    mesh = Mesh(np.asarray(jax.devices()[:n_cores]), ("core",))
    in_specs = (PartitionSpec("core"),) * (n_params + len(out_names))
    out_specs = (PartitionSpec("core"),) * len(out_names)
    jf = jax.jit(
        shard_map(_body, mesh=mesh, in_specs=in_specs, out_specs=out_specs,
                  check_rep=False),
        donate_argnums=donate, keep_unused=True)

    import hashlib
    dev_cache = {}

    def _zeros_dev():
        return [jax.numpy.zeros((n_cores * shape[0],) + shape[1:], dtype)
                for shape, dtype in zero_shapes]

    def run(in_maps):
        h = hashlib.md5()
        for name in in_names:
            for m in in_maps:
                h.update(np.asarray(m[name]).tobytes())
        key = h.hexdigest()
        if key not in dev_cache:
            dev_cache.clear()
            arrs = [np.concatenate([np.asarray(m[name]) for m in in_maps], axis=0)
                    for name in in_names]
            dev_cache[key] = [jax.device_put(a) for a in arrs]
        args = list(dev_cache[key]) + _zeros_dev()
        outs = jf(*args)
        return [{n: np.asarray(outs[i]).reshape((n_cores,) + zero_shapes[i][0])[c]
                 for i, n in enumerate(out_names)}
                for c in range(n_cores)]

    return run


def kernel(**inputs):
    """DGCNN forward. inputs keyed as reference.setup_inputs(); returns
    (B, 63, N) float32. Data-parallel: one NeuronCore per sample."""
    from concourse.bass_utils import run_bass_kernel_spmd

    x = np.asarray(inputs["x"])
    B, _, N = x.shape
    key = (B, N)
    if key not in _CACHE:
        nc = build_core(N=N, conv_dtype="fp16", sel_mode="fast2")
        runner = None
        try:
            runner = _make_runner(nc, B)
        except Exception:
            runner = None
        _CACHE[key] = (nc, runner)
    nc, runner = _CACHE[key]
    in_maps = cast_inputs(make_in_maps(inputs, n_cores=B), nc)
    if runner is not None:
        try:
            results = runner(in_maps)
            return np.stack([results[b]["out"] for b in range(B)]).astype(np.float32)
        except Exception:
            _CACHE[key] = (nc, None)
    res = run_bass_kernel_spmd(nc, in_maps, core_ids=list(range(B)))
    return np.stack([res.results[b]["out"] for b in range(B)]).astype(np.float32)

